# revision 1
# baseline (speedup 1.0000x reference)
"""Trainium2 Bass kernel for 3D deformable attention (8 NeuronCores).

Sharding: core c handles (b, g) = (c // 4, c % 4): batch b, group g
(= heads 2g, 2g+1).  Each core runs the full offset/sampling branch for
all 4 groups of its batch (v1: replicated), attention for its own two
heads over all 4096 queries, and a partial output projection
y_partial = wo[:, 32g:32g+32] @ out_heads.  The host sums the four
partials per batch and adds bo.

The program is SPMD (one compiled NEFF for all cores); all per-core
variation is carried in the input tensors.  Q-projection output channels
are permuted per core so the core's own group block lands at partitions
96..127 (blocks 0..3 = permuted groups, own group last).

Numerical notes vs the jax reference:
 - bk is dropped: a per-(head,query) constant shift of attention logits
   is softmax-invariant.
 - bv enters as wo[:, hs] @ bv[hs] added to the partial output.
 - softmax skips the max-subtraction (logits are O(0.3)).
 - gelu(exact-erf) is replaced by the tanh approximation, with tanh and
   LayerNorm's rsqrt computed from exp/ln so one ACT table set serves
   the whole kernel.
"""

import math
import sys

for _p in ("/opt/trn_rl_repo",):
    if _p not in sys.path:
        sys.path.insert(0, _p)

import numpy as np

import concourse.bass as bass
import concourse.mybir as mybir
import concourse.tile as tile
from concourse import bacc
from concourse.masks import make_identity

F32 = mybir.dt.float32
F32R = mybir.dt.float32r
I32 = mybir.dt.int32
I16 = mybir.dt.int16
AF = mybir.ActivationFunctionType
ALU = mybir.AluOpType

B = 2
CH = 128
HEADS = 8
GROUPS = 4
GC = CH // GROUPS     # 32
HC = CH // HEADS      # 16
SP = 16
NQ = SP * SP * SP     # 4096
DK = 8
NS = DK * DK * DK     # 512 samples per group
KS = 3
EPS = 1e-5
SCALE = HC ** -0.5
XSLOTS = SP + 2       # x slots represent x = -1 .. 16 (18 slots)
ZYROWS = SP * SP      # 256
G_ROWS = ZYROWS * XSLOTS   # 4608 gather rows per group
N_IDX = GROUPS * 4 * NS    # 8192 gather descriptors
GELU_C = 0.044715
GELU_S = math.sqrt(2.0 / math.pi)


# ============================================================ host prep

def _np(x):
    return np.ascontiguousarray(np.asarray(x, dtype=np.float32))


def host_prep(inp):
    """inp: dict of full numpy inputs. Returns (in_maps, bo)."""
    Qf = _np(inp["Q_feature"])
    KVf = _np(inp["KV_feature"])
    wq = _np(inp["wq"]); bq = _np(inp["bq"])
    w_off_dw = _np(inp["w_off_dw"]); b_off_dw = _np(inp["b_off_dw"])
    ln_w = _np(inp["ln_w"]); ln_b = _np(inp["ln_b"])
    w_off_proj = _np(inp["w_off_proj"])
    wk = _np(inp["wk"]); wv = _np(inp["wv"]); bv = _np(inp["bv"])
    wo = _np(inp["wo"])

    # ---- gather source (per batch): rows (g, zy, xslot) of 64 floats:
    # [KV[c, z, y, x(slot)], KV[c, z, y, x(slot)+1]], x padded (-1..17).
    kvt_b = []
    for b in range(B):
        kv = KVf[b].reshape(GROUPS, GC, SP, SP, SP)
        kvp = np.zeros((GROUPS, GC, SP, SP, SP + 3), np.float32)
        kvp[..., 1:SP + 1] = kv
        a0 = kvp[..., 0:XSLOTS]
        a1 = kvp[..., 1:XSLOTS + 1]
        st = np.stack([a0, a1], axis=-1)          # [G, GC, Z, Y, XS, 2]
        st = st.transpose(0, 2, 3, 4, 5, 1)       # [G, Z, Y, XS, 2, GC]
        kvt_b.append(np.ascontiguousarray(st.reshape(GROUPS * G_ROWS, 2 * GC)))

    qf_b = [np.ascontiguousarray(Qf[b].reshape(CH, NQ)) for b in range(B)]

    # mean / bcast lhsT are block-structured (permutation-invariant)
    mean_lhsT = np.zeros((CH, GROUPS), np.float32)
    bcast_lhsT = np.zeros((GROUPS, CH), np.float32)
    for j in range(GROUPS):
        mean_lhsT[j * GC:(j + 1) * GC, j] = 1.0 / GC
        bcast_lhsT[j, j * GC:(j + 1) * GC] = 1.0

    # sampling reference grid (z, y, x), s = Z*64 + Y*8 + X
    r = (np.linspace(0.5, DK - 0.5, DK, dtype=np.float32) / DK) * 2 - 1
    zz, yy, xx = np.meshgrid(r, r, r, indexing="ij")
    axes = [zz.reshape(NS), yy.reshape(NS), xx.reshape(NS)]
    rxyz = np.zeros((12, NS), np.float32)
    for ax in range(3):
        for j in range(GROUPS):
            rxyz[ax * 4 + j] = (axes[ax] + 1.0) * 7.5 + 1.875

    wdw = w_off_dw.reshape(GC, KS, KS, KS)
    in_maps = []
    for c in range(HEADS):
        b, g = c // GROUPS, c % GROUPS
        # block j = group j (no permutation needed)
        border = list(range(GROUPS))
        # permuted channel list: block j holds group border[j]'s channels
        pch = np.concatenate([np.arange(gg * GC, (gg + 1) * GC)
                              for gg in border])

        wq_t = np.ascontiguousarray(wq[pch, :].T)       # [128 in, 128 out-perm]
        bq_c = bq[pch].reshape(CH, 1)

        convw = np.zeros((KS ** 3, CH, CH), np.float32)
        for dz in range(KS):
            for dy in range(KS):
                for dx in range(KS):
                    t = (dz * KS + dy) * KS + dx
                    d = np.tile(wdw[:, dz, dy, dx], GROUPS)  # per-channel,
                    # same for every group and thus permutation-invariant
                    convw[t][np.arange(CH), np.arange(CH)] = d
        bdw_c = np.tile(b_off_dw, GROUPS).reshape(CH, 1)
        lnw_c = np.tile(ln_w, GROUPS).reshape(CH, 1)
        lnb_c = np.tile(ln_b, GROUPS).reshape(CH, 1)

        projw_neg = np.zeros((CH, 12), np.float32)
        for j in range(GROUPS):
            for ax in range(3):
                projw_neg[j * GC:(j + 1) * GC, ax * 4 + j] = -w_off_proj[ax]

        goff = np.zeros((GROUPS, 1), np.float32)
        for j in range(GROUPS):
            goff[j] = 1.0 + border[j] * G_ROWS

        # attention-side weights; xs channel space is (block j, c) =
        # original channel pch
        hs = slice(g * GC, (g + 1) * GC)
        wk_h = [np.ascontiguousarray(
            (wk[g * GC + h * HC: g * GC + (h + 1) * HC, :][:, pch] * SCALE).T)
            for h in range(2)]
        wq_h = [np.ascontiguousarray(
            wq[g * GC + h * HC: g * GC + (h + 1) * HC, :].T)
            for h in range(2)]
        bq_h = [bq[g * GC + h * HC: g * GC + (h + 1) * HC].reshape(HC, 1)
                for h in range(2)]
        wv_t = np.ascontiguousarray(wv[hs, :][:, pch].T)     # [128, 32]

        wo_s = []
        for mh in range(2):
            m = np.zeros((CH, CH), np.float32)
            for h in range(2):
                j = h * 2 + mh
                cols = wo[:, g * GC + h * HC: g * GC + (h + 1) * HC]
                m[32 * j + 1: 32 * j + 17, :] = cols.T
            wo_s.append(np.ascontiguousarray(m))
        bc4 = np.zeros((GROUPS, CH), np.float32)
        for j in range(GROUPS):
            bc4[j, 32 * j + 1: 32 * j + 17] = 1.0
        ybias = (wo[:, hs] @ bv[hs]).reshape(CH, 1)

        in_maps.append({
            "qf": qf_b[b], "kvt": kvt_b[b],
            "wq_t": wq_t, "bq_c": bq_c,
            "convw": convw, "bdw_c": bdw_c,
            "lnw_c": lnw_c, "lnwn_c": -lnw_c, "lnb_c": lnb_c,
            "mean_lhsT": mean_lhsT, "bcast_lhsT": bcast_lhsT,
            "projw_neg": projw_neg, "rxyz": rxyz, "goff": goff,
            "wk_h0": wk_h[0], "wk_h1": wk_h[1], "wv_t": wv_t,
            "wq_h0": wq_h[0], "wq_h1": wq_h[1],
            "bq_h0": bq_h[0], "bq_h1": bq_h[1],
            "wo_sA": wo_s[0], "wo_sB": wo_s[1], "bc4": bc4,
            "ybias": ybias,
        })
    return in_maps


def host_post(results, bo):
    """results: list of 8 dicts with 'py' [128, 4096]."""
    bo = _np(bo)
    y = np.zeros((B, CH, NQ), np.float32)
    for c in range(HEADS):
        y[c // GROUPS] += results[c]["py"]
    y += bo.reshape(1, CH, 1)
    return y.reshape(B, CH, SP, SP, SP)


# ============================================================ device build

def build_program(tc: tile.TileContext, ctx):
    nc = tc.nc

    def dram_in(name, shape, dt=F32):
        return nc.dram_tensor(name, list(shape), dt, kind="ExternalInput").ap()

    qf = dram_in("qf", (CH, NQ))
    kvt = dram_in("kvt", (GROUPS * G_ROWS, 2 * GC))
    wq_t = dram_in("wq_t", (CH, CH))
    bq_c = dram_in("bq_c", (CH, 1))
    convw = dram_in("convw", (KS ** 3, CH, CH))
    bdw_c = dram_in("bdw_c", (CH, 1))
    lnw_c = dram_in("lnw_c", (CH, 1))
    lnwn_c = dram_in("lnwn_c", (CH, 1))
    lnb_c = dram_in("lnb_c", (CH, 1))
    mean_l = dram_in("mean_lhsT", (CH, GROUPS))
    bcast_l = dram_in("bcast_lhsT", (GROUPS, CH))
    projw = dram_in("projw_neg", (CH, 12))
    rxyz = dram_in("rxyz", (12, NS))
    goff = dram_in("goff", (GROUPS, 1))
    wq_h0 = dram_in("wq_h0", (CH, HC))
    wq_h1 = dram_in("wq_h1", (CH, HC))
    bq_h0 = dram_in("bq_h0", (HC, 1))
    bq_h1 = dram_in("bq_h1", (HC, 1))
    wk_h0 = dram_in("wk_h0", (CH, HC))
    wk_h1 = dram_in("wk_h1", (CH, HC))
    wv_t = dram_in("wv_t", (CH, GC))
    wo_sA = dram_in("wo_sA", (CH, CH))
    wo_sB = dram_in("wo_sB", (CH, CH))
    bc4 = dram_in("bc4", (GROUPS, CH))
    ybias = dram_in("ybias", (CH, 1))

    py = nc.dram_tensor("py", [CH, NQ], F32, kind="ExternalOutput").ap()

    idx_dram = nc.dram_tensor("idx_dram", [N_IDX], I16).ap()
    co_dram = nc.dram_tensor("co_dram", [12 * 3 * NS], F32).ap()
    w8_dram = nc.dram_tensor("w8_dram", [64 * 2 * CH], F32).ap()

    consts = ctx.enter_context(tc.tile_pool(name="consts", bufs=1))
    live = ctx.enter_context(tc.tile_pool(name="live", bufs=1))

    def load(ap, name, pool=consts, shape=None, dt=F32):
        t = pool.tile(list(shape or ap.shape), dt, tag=name, name=name)
        nc.sync.dma_start(t[:], ap)
        return t

    wq_sb = load(wq_t, "wq_sb")
    bq_sb = load(bq_c, "bq_sb")
    bdw_sb = load(bdw_c, "bdw_sb")
    lnw_sb = load(lnw_c, "lnw_sb")
    lnwn_sb = load(lnwn_c, "lnwn_sb")
    lnb_sb = load(lnb_c, "lnb_sb")
    mean_sb = load(mean_l, "mean_sb")
    bcast_sb = load(bcast_l, "bcast_sb")
    projw_sb = load(projw, "projw_sb")
    rxyz_sb = load(rxyz, "rxyz_sb")
    goff_sb = load(goff, "goff_sb")
    wq0_sb = load(wq_h0, "wq0_sb")
    wq1_sb = load(wq_h1, "wq1_sb")
    bq0_sb = load(bq_h0, "bq0_sb")
    bq1_sb = load(bq_h1, "bq1_sb")
    wk0_sb = load(wk_h0, "wk0_sb")
    wk1_sb = load(wk_h1, "wk1_sb")
    wv_sb = load(wv_t, "wv_sb")
    woA_sb = load(wo_sA, "woA_sb")
    woB_sb = load(wo_sB, "woB_sb")
    bc4_sb = load(bc4, "bc4_sb")
    ybias_sb = load(ybias, "ybias_sb")

    convw_sb = consts.tile([CH, KS ** 3 * CH], F32, tag="convw_sb", name="convw_sb")
    nc.sync.dma_start(convw_sb[:].rearrange("p (t c) -> p t c", t=KS ** 3),
                      convw.rearrange("t p c -> p t c"))

    ident = consts.tile([CH, CH], F32, tag="ident", name="ident")
    make_identity(nc, ident[:])

    # tiles that outlive the scratch phases
    q2_sb = live.tile([HC, 2 * NQ], F32, tag="q2_sb", name="q2_sb")
    k2_sb = live.tile([HC, 2 * NS], F32, tag="k2_sb", name="k2_sb")
    vt_sb = live.tile([CH, 4 * 64], F32, tag="vt_sb", name="vt_sb")
    xs_sb = live.tile([CH, GROUPS * CH], F32, tag="xs_sb", name="xs_sb")

    USE_F32R = False

    def r32(ap):
        return ap.bitcast(F32R) if USE_F32R else ap

    with tc.tile_pool(name="scratch", bufs=1) as scr, \
         tc.tile_pool(name="pq", bufs=2, space="PSUM") as pq_pool, \
         tc.tile_pool(name="pst", bufs=1, space="PSUM") as pst_pool:

        _cnt = [0]

        def st(tag, shape=(CH, NS), dt=F32):
            _cnt[0] += 1
            return scr.tile(list(shape), dt, tag=tag,
                            name=f"{tag}_{_cnt[0]}")

        # ---- phase 1: Q projection into a zero-padded 18^3 buffer ----
        qf_sb = st("tA", (CH, NQ))
        nc.sync.dma_start(qf_sb[:], qf)
        SPP = SP + 1
        q_pad = st("qpad", (CH, SPP ** 3))
        nc.gpsimd.memset(q_pad[:], 0.0)
        qp_zyx = q_pad[:].rearrange("p (z y x) -> p z y x", z=SPP, y=SPP)
        for i in range(NQ // 512):   # chunk = 2 z-slabs
            pq = pq_pool.tile([CH, 512], F32, tag="pq", name="pq")
            nc.tensor.matmul(pq[:], r32(wq_sb[:]),
                             r32(qf_sb[:, i * 512:(i + 1) * 512]),
                             start=True, stop=True)
            nc.scalar.activation(
                qp_zyx[:, 1 + 2 * i:3 + 2 * i, 1:SP + 1, 1:SP + 1],
                pq[:].rearrange("p (a b c) -> p a b c", a=2, b=SP),
                AF.Identity, bias=bq_sb[:])
        # per-head Q rows for attention (own group's two heads)
        for h, (wqh, bqh) in enumerate([(wq0_sb, bq0_sb), (wq1_sb, bq1_sb)]):
            for i in range(NQ // 512):
                pq2 = pst_pool.tile([HC, 512], F32, tag="pq2", name="pq2")
                nc.tensor.matmul(pq2[:], r32(wqh[:]),
                                 r32(qf_sb[:, i * 512:(i + 1) * 512]),
                                 start=True, stop=True)
                nc.vector.tensor_scalar(
                    q2_sb[:, h * NQ + i * 512:h * NQ + (i + 1) * 512],
                    pq2[:], bqh[:], None, ALU.add)

        # ---- phase 2: depthwise conv (stride 2) ----------------------
        pc = pst_pool.tile([CH, NS], F32, tag="psA", name="psA")
        first = True
        for dz in range(KS):
            for dy in range(KS):
                for dx in range(KS):
                    t = (dz * KS + dy) * KS + dx
                    rhs = qp_zyx[:, dz:dz + 2 * DK - 1:2,
                                 dy:dy + 2 * DK - 1:2,
                                 dx:dx + 2 * DK - 1:2]
                    nc.tensor.matmul(pc[:], r32(convw_sb[:, t * CH:(t + 1) * CH]),
                                     r32(rhs), start=first,
                                     stop=(t == KS ** 3 - 1))
                    first = False
        c_sb = st("s0")
        nc.scalar.activation(c_sb[:], pc[:], AF.Identity, bias=bdw_sb[:])

        # ---- phase 3: LayerNorm over 32-channel blocks ---------------
        csq = st("s1")
        nc.scalar.activation(csq[:], c_sb[:], AF.Square)
        pmu = pst_pool.tile([GROUPS, NS], F32, tag="psB", name="psB")
        nc.tensor.matmul(pmu[:], r32(mean_sb[:]), r32(c_sb[:]),
                         start=True, stop=True)
        pmsq = pst_pool.tile([GROUPS, NS], F32, tag="psC", name="psC")
        nc.tensor.matmul(pmsq[:], r32(mean_sb[:]), r32(csq[:]),
                         start=True, stop=True)
        mu2 = st("s2", (GROUPS, NS))
        nc.scalar.activation(mu2[:], pmu[:], AF.Square)
        var = st("s3", (GROUPS, NS))
        nc.vector.tensor_sub(var[:], pmsq[:], mu2[:])
        eps_sb = st("eps", (GROUPS, 1))
        nc.vector.memset(eps_sb[:], EPS)
        lnv = st("s2b", (GROUPS, NS))
        nc.scalar.activation(lnv[:], var[:], AF.Ln, bias=eps_sb[:])
        rstd = st("s1b", (GROUPS, NS))
        nc.scalar.activation(rstd[:], lnv[:], AF.Exp, scale=-0.5)
        murstd = st("s3b", (GROUPS, NS))
        nc.vector.tensor_mul(murstd[:], pmu[:], rstd[:])
        prb = pst_pool.tile([CH, NS], F32, tag="psB2", name="psB2")
        nc.tensor.matmul(prb[:], r32(bcast_sb[:]), r32(rstd[:]),
                         start=True, stop=True)
        pmb = pst_pool.tile([CH, NS], F32, tag="psC2", name="psC2")
        nc.tensor.matmul(pmb[:], r32(bcast_sb[:]), r32(murstd[:]),
                         start=True, stop=True)
        a_bc = st("s2")
        nc.vector.tensor_scalar(a_bc[:], prb[:], lnw_sb[:], None, ALU.mult)
        b_bc = st("s3")
        nc.vector.tensor_scalar(b_bc[:], pmb[:], lnwn_sb[:], lnb_sb[:],
                                ALU.mult, ALU.add)
        u = st("s4")
        nc.vector.tensor_mul(u[:], c_sb[:], a_bc[:])
        nc.vector.tensor_add(u[:], u[:], b_bc[:])

        # ---- phase 4: gelu (tanh approx; tanh via exp) ---------------
        usq = st("s0")
        nc.scalar.activation(usq[:], u[:], AF.Square)
        ucb = st("s1")
        nc.vector.tensor_mul(ucb[:], usq[:], u[:])
        g2 = st("s2")
        nc.vector.scalar_tensor_tensor(g2[:], ucb[:], GELU_C, u[:],
                                       ALU.mult, ALU.add)
        ge = st("s3")
        nc.scalar.activation(ge[:], g2[:], AF.Exp, scale=2.0 * GELU_S)
        nc.vector.tensor_scalar(ge[:], ge[:], 1.0, None, ALU.add)
        gr = st("s0")
        nc.vector.reciprocal(gr[:], ge[:])
        gneg = st("s1")
        nc.vector.scalar_tensor_tensor(gneg[:], gr[:], 1.0, u[:],
                                       ALU.subtract, ALU.mult)  # -gelu

        # ---- phase 5: offset proj + coords ---------------------------
        poff = pst_pool.tile([12, NS], F32, tag="psB", name="psB")
        nc.tensor.matmul(poff[:], r32(projw_sb[:]), r32(gneg[:]),
                         start=True, stop=True)
        ce = st("s2", (12, NS))
        nc.scalar.activation(ce[:], poff[:], AF.Exp, scale=2.0)
        nc.vector.tensor_scalar(ce[:], ce[:], 1.0, None, ALU.add)
        cr = st("s3", (12, NS))
        nc.vector.reciprocal(cr[:], ce[:])
        ixyz = st("s4", (12, NS))
        nc.vector.scalar_tensor_tensor(ixyz[:], cr[:], -3.75, rxyz_sb[:],
                                       ALU.mult, ALU.add)
        ci = st("s0", (12, NS), I32)
        nc.vector.tensor_copy(ci[:], ixyz[:])
        cf = st("s1", (12, NS))
        nc.vector.tensor_copy(cf[:], ci[:])
        fixm = st("s2", (12, NS))
        nc.vector.tensor_tensor(fixm[:], cf[:], ixyz[:], ALU.is_gt)
        f0 = st("s5", (12, NS))
        nc.vector.tensor_sub(f0[:], cf[:], fixm[:])
        tfrac = st("s3", (12, NS))
        nc.vector.tensor_sub(tfrac[:], ixyz[:], f0[:])
        m0 = st("s0", (12, NS))
        nc.vector.tensor_scalar(m0[:], f0[:], 0.0, None, ALU.is_ge)
        m1 = st("s1", (12, NS))
        nc.vector.tensor_scalar(m1[:], f0[:], 14.0, None, ALU.is_le)
        omt = st("s2", (12, NS))
        nc.vector.tensor_scalar(omt[:], tfrac[:], -1.0, 1.0, ALU.mult, ALU.add)

        big = st("big", (12, 3 * NS))
        nc.vector.tensor_copy(big[:, 0:NS], f0[:])
        nc.vector.tensor_mul(big[:, NS:2 * NS], omt[:], m0[:])
        nc.vector.tensor_mul(big[:, 2 * NS:3 * NS], tfrac[:], m1[:])
        nc.sync.dma_start(co_dram, big[:])
        co_g = st("co_g", (GROUPS, 9 * NS))
        nc.sync.dma_start(
            co_g[:].rearrange("g (ax k s) -> g ax k s", ax=3, k=3),
            co_dram.rearrange("(ax g k s) -> g ax k s", ax=3, g=4, k=3))

        def cgs(ax, kind):  # kind: 0 = floor, 1 = w0, 2 = w1
            o = (ax * 3 + kind) * NS
            return co_g[:, o:o + NS]

        zc0 = st("s0", (GROUPS, NS))
        zc1 = st("s1", (GROUPS, NS))
        yc0 = st("s2", (GROUPS, NS))
        yc1 = st("s3", (GROUPS, NS))
        nc.vector.tensor_scalar(zc0[:], cgs(0, 0), 0.0, 15.0, ALU.max, ALU.min)
        nc.vector.tensor_scalar(zc1[:], cgs(0, 0), 1.0, 0.0, ALU.add, ALU.max)
        nc.vector.tensor_scalar(zc1[:], zc1[:], 15.0, None, ALU.min)
        nc.vector.tensor_scalar(yc0[:], cgs(1, 0), 0.0, 15.0, ALU.max, ALU.min)
        nc.vector.tensor_scalar(yc1[:], cgs(1, 0), 1.0, 0.0, ALU.add, ALU.max)
        nc.vector.tensor_scalar(yc1[:], yc1[:], 15.0, None, ALU.min)
        xoff2 = st("s4", (GROUPS, NS))
        nc.vector.tensor_scalar(xoff2[:], cgs(2, 0), goff_sb[:], None, ALU.add)

        idxf = st("s5", (GROUPS, NS))
        idx16 = st("idx16", (GROUPS, 4 * NS), I16)
        wzy = st("wzy", (GROUPS, 4 * NS))
        zcs, ycs = [zc0, zc1], [yc0, yc1]
        for a in range(2):
            for bb in range(2):
                zy = a * 2 + bb
                nc.vector.scalar_tensor_tensor(
                    idxf[:], zcs[a][:], float(SP * XSLOTS), xoff2[:],
                    ALU.mult, ALU.add)
                nc.vector.scalar_tensor_tensor(
                    idxf[:], ycs[bb][:], float(XSLOTS), idxf[:],
                    ALU.mult, ALU.add)
                nc.vector.tensor_scalar(idxf[:], idxf[:], 0.0,
                                        float(GROUPS * G_ROWS - 1),
                                        ALU.max, ALU.min)
                nc.vector.tensor_copy(idx16[:, zy * NS:(zy + 1) * NS], idxf[:])
                nc.vector.tensor_mul(wzy[:, zy * NS:(zy + 1) * NS],
                                     cgs(0, 1 + a), cgs(1, 1 + bb))
        nc.sync.dma_start(idx_dram, idx16[:])
        # full trilinear corner weights w8[g, zy, x, s] = wzy * wx,
        # stored x-interleaved: [g, zy, s, x]
        w8s = st("w8s", (GROUPS, 4 * 2 * NS))
        w8sv = w8s[:].rearrange("g (zy s x) -> g zy s x", zy=4, x=2)
        for zy in range(4):
            for x in range(2):
                nc.vector.tensor_mul(w8sv[:, zy, :, x],
                                     wzy[:, zy * NS:(zy + 1) * NS],
                                     cgs(2, 1 + x))
        # bounce to DRAM with addressing (g zy si p x) = (j, p, x)
        w8d = w8_dram.rearrange("(g zy si p x) -> g zy si p x",
                                g=4, zy=4, si=4, x=2)
        for zy in range(4):
            nc.sync.dma_start(
                w8d[:, zy].rearrange("g si p x -> g (si p x)"),
                w8sv[:, zy].rearrange("g s x -> g (s x)"))

        # wrapped idx [128, 512]: global idx i at (i%16, i//16), x8 blocks
        import os as _os
        idxw = st("idxw", (CH, N_IDX // 16), I16)
        if _os.environ.get("DEFORM_NO_IDXW"):
            nc.vector.memset(idxw[:], 0)
        else:
            for rep in range(8):
                nc.gpsimd.dma_start(
                    idxw[rep * 16:(rep + 1) * 16, :],
                    idx_dram.rearrange("(col r) -> r col", r=16))

        # ---- phase 6: gather + trilinear combine ---------------------
        gth = scr.tile([CH, N_IDX // CH, 2 * GC], F32, tag="tA", name="tA")
        if _os.environ.get("DEFORM_NO_GATHER"):
            nc.vector.memset(gth[:], 0.25)
        else:
            NCHK = 32
            CH_I = N_IDX // NCHK          # 256 idx per gather call
            for k in range(NCHK):
                nc.gpsimd.dma_gather(
                    out_ap=gth[:, k * (CH_I // CH) * ...] if False else
                    gth[:, k * (CH_I // 128):(k + 1) * (CH_I // 128), :],
                    in_ap=kvt,
                    idxs_ap=idxw[:, k * (CH_I // 16):(k + 1) * (CH_I // 16)],
                    num_idxs=CH_I, num_idxs_reg=CH_I, elem_size=2 * GC)

        # stream order: i = ((g*4 + zy)*4 + si)*128 + p, sample s = si*128+p
        w8b = scr.tile([CH, 64, 2], F32, tag="tB", name="w8b")
        nc.sync.dma_start(
            w8b[:],
            w8_dram.rearrange("(j p x) -> p j x", j=64, x=2))
        t2 = scr.tile([CH, 64, 2 * GC], F32, tag="tC", name="t2")
        nc.vector.tensor_tensor(
            t2[:].rearrange("p j (x c) -> p j x c", x=2),
            gth[:].rearrange("p j (x c) -> p j x c", x=2),
            w8b[:].unsqueeze(3).broadcast_to([CH, 64, 2, GC]), ALU.mult)
        t2v = t2[:].rearrange("p (g zy si) e -> p g zy (si e)", g=4, zy=4)
        sa = st("sa", (CH, GROUPS, 4 * 2 * GC))
        sb = st("sb", (CH, GROUPS, 4 * 2 * GC))
        nc.vector.tensor_tensor(sa[:], t2v[:, :, 0], t2v[:, :, 1], ALU.add)
        nc.vector.tensor_tensor(sb[:], t2v[:, :, 2], t2v[:, :, 3], ALU.add)
        nc.vector.tensor_tensor(sa[:], sa[:], sb[:], ALU.add)
        sav = sa[:].rearrange("p g (si x c) -> p g si x c", si=4, x=2)
        xs_t = st("s0", (CH, 4, GROUPS, GC))   # [p, si, g, c]
        nc.vector.tensor_tensor(xs_t[:].rearrange("p si g c -> p g si c"),
                                sav[:, :, :, 0, :],
                                sav[:, :, :, 1, :], ALU.add)

        # ---- phase 7: transpose to xs [128 (blk,c), 512 n] -----------
        for si in range(4):
            pt = pst_pool.tile([CH, CH], F32, tag="psB", name="psB")
            nc.tensor.transpose(
                pt[:], xs_t[:, si].rearrange("p g c -> p (g c)"), ident[:])
            nc.scalar.activation(xs_sb[:, si * CH:(si + 1) * CH], pt[:],
                                 AF.Identity)

        # ---- phase 8: K and V-hat ------------------------------------
        for h, wkh in enumerate([wk0_sb, wk1_sb]):
            pk = pst_pool.tile([HC, NS], F32, tag="psC", name="psC")
            nc.tensor.matmul(pk[:], r32(wkh[:]), r32(xs_sb[:]),
                             start=True, stop=True)
            nc.scalar.activation(k2_sb[:, h * NS:(h + 1) * NS], pk[:],
                                 AF.Identity)
        nc.vector.memset(vt_sb[:], 0.0)
        nc.vector.memset(
            vt_sb[:].rearrange("p (n h s) -> p n h s", n=4, h=2)[:, :, :, 0:1],
            1.0)
        for nch in range(4):
            pv = pst_pool.tile([CH, GC], F32, tag="psA", name="psA")
            nc.tensor.matmul(pv[:], r32(xs_sb[:, nch * CH:(nch + 1) * CH]),
                             r32(wv_sb[:]), start=True, stop=True)
            nc.vector.tensor_copy(
                vt_sb[:].rearrange("p (n h s) -> p n h s", n=4, h=2)
                [:, nch, :, 1:17],
                pv[:].rearrange("p (h c) -> p h c", h=2))
        # (vt slot layout per n-chunk: [1 | V(16) | 0*15] x 2 heads, 64 wide)

    # ---- phase 9: attention loop -------------------------------------
    with tc.tile_pool(name="pA", bufs=2, space="PSUM") as pA, \
         tc.tile_pool(name="pO", bufs=2, space="PSUM") as pO, \
         tc.tile_pool(name="pR", bufs=1, space="PSUM") as pR, \
         tc.tile_pool(name="pY", bufs=1, space="PSUM") as pY, \
         tc.tile_pool(name="att_pool", bufs=3) as att_pool, \
         tc.tile_pool(name="opool", bufs=2) as opool:
        for mq in range(4):
            po = pO.tile([CH, 512], F32, tag="po", name="po")
            for h in range(2):
                for nch in range(4):
                    pa = pA.tile([CH, 1024], F32, tag="pa", name="pa")
                    for mh in range(2):
                        nc.tensor.matmul(
                            pa[:, mh * 512:(mh + 1) * 512],
                            r32(k2_sb[:, h * NS + nch * CH:
                                      h * NS + (nch + 1) * CH]),
                            r32(q2_sb[:, h * NQ + mq * 1024 + mh * 512:
                                      h * NQ + mq * 1024 + (mh + 1) * 512]),
                            start=True, stop=True)
                    att = att_pool.tile([CH, 1024], F32, tag="att", name="att")
                    nc.scalar.activation(att[:], pa[:], AF.Exp)
                    for mh in range(2):
                        j = h * 2 + mh
                        nc.tensor.matmul(
                            po[32 * j:32 * j + 32, :],
                            r32(vt_sb[:, nch * 64 + h * 32:
                                      nch * 64 + (h + 1) * 32]),
                            r32(att[:, mh * 512:(mh + 1) * 512]),
                            start=(nch == 0), stop=(nch == 3),
                            skip_group_check=True,
                            tile_position=(0, 32 * j))
            o_sb = opool.tile([CH, 512], F32, tag="o_sb", name="o_sb")
            nc.scalar.activation(o_sb[:], po[:], AF.Identity)
            den4 = opool.tile([GROUPS, 512], F32, tag="den4", name="den4")
            for j in range(4):
                nc.sync.dma_start(den4[j:j + 1, :], o_sb[32 * j:32 * j + 1, :])
            rd4 = opool.tile([GROUPS, 512], F32, tag="rd4", name="rd4")
            nc.vector.reciprocal(rd4[:], den4[:])
            prd = pR.tile([CH, 512], F32, tag="prd", name="prd")
            nc.tensor.matmul(prd[:], r32(bc4_sb[:]), r32(rd4[:]),
                             start=True, stop=True)
            on_sb = opool.tile([CH, 512], F32, tag="on_sb", name="on_sb")
            nc.vector.tensor_mul(on_sb[:], o_sb[:], prd[:])
            for mh, wos in enumerate([woA_sb, woB_sb]):
                pyp = pY.tile([CH, 512], F32, tag="pyp", name="pyp")
                nc.tensor.matmul(pyp[:], r32(wos[:]), r32(on_sb[:]),
                                 start=True, stop=True)
                y_sb = opool.tile([CH, 512], F32, tag="y_sb", name="y_sb")
                nc.scalar.activation(y_sb[:], pyp[:], AF.Identity,
                                     bias=ybias_sb[:])
                nc.sync.dma_start(
                    py[:, mq * 1024 + mh * 512:mq * 1024 + (mh + 1) * 512],
                    y_sb[:])


# ============================================================ entry points

_CACHE = {}


def _get_compiled():
    if "nc" in _CACHE:
        return _CACHE["nc"]
    from contextlib import ExitStack
    nc = bacc.Bacc("TRN2", target_bir_lowering=False, debug=False,
                   num_devices=HEADS)
    with tile.TileContext(nc) as tc:
        with ExitStack() as ctx:
            build_program(tc, ctx)
    nc.compile()
    _CACHE["nc"] = nc
    return nc


def kernel(**inputs):
    from concourse.bass_utils import run_bass_kernel_spmd
    nc = _get_compiled()
    in_maps = host_prep(inputs)
    res = run_bass_kernel_spmd(nc, in_maps, list(range(HEADS)))
    return host_post(res.results, inputs["bo"])


if __name__ == "__main__":
    _get_compiled()
    print("build + compile OK")



# revision 2
# speedup vs baseline: 1.8514x; 1.8514x over previous
"""Trainium2 Bass kernel for 3D deformable attention (8 NeuronCores).

Sharding: core c handles (b, g) = (c // 4, c % 4): batch b, group g
(= heads 2g, 2g+1).  Each core runs the full offset/sampling branch for
all 4 groups of its batch (v1: replicated), attention for its own two
heads over all 4096 queries, and a partial output projection
y_partial = wo[:, 32g:32g+32] @ out_heads.  The host sums the four
partials per batch and adds bo.

The program is SPMD (one compiled NEFF for all cores); all per-core
variation is carried in the input tensors.  Q-projection output channels
are permuted per core so the core's own group block lands at partitions
96..127 (blocks 0..3 = permuted groups, own group last).

Numerical notes vs the jax reference:
 - bk is dropped: a per-(head,query) constant shift of attention logits
   is softmax-invariant.
 - bv enters as wo[:, hs] @ bv[hs] added to the partial output.
 - softmax skips the max-subtraction (logits are O(0.3)).
 - gelu(exact-erf) is replaced by the tanh approximation, with tanh and
   LayerNorm's rsqrt computed from exp/ln so one ACT table set serves
   the whole kernel.
"""

import math
import sys

for _p in ("/opt/trn_rl_repo",):
    if _p not in sys.path:
        sys.path.insert(0, _p)

import numpy as np

import concourse.bass as bass
import concourse.mybir as mybir
import concourse.tile as tile
from concourse import bacc
from concourse.masks import make_identity

F32 = mybir.dt.float32
F32R = mybir.dt.float32r
I32 = mybir.dt.int32
I16 = mybir.dt.int16
AF = mybir.ActivationFunctionType
ALU = mybir.AluOpType

B = 2
CH = 128
HEADS = 8
GROUPS = 4
GC = CH // GROUPS     # 32
HC = CH // HEADS      # 16
SP = 16
NQ = SP * SP * SP     # 4096
DK = 8
NS = DK * DK * DK     # 512 samples per group
KS = 3
EPS = 1e-5
SCALE = HC ** -0.5
XSLOTS = SP + 2       # x slots represent x = -1 .. 16 (18 slots)
ZYROWS = SP * SP      # 256
G_ROWS = ZYROWS * XSLOTS   # 4608 gather rows per group
N_IDX = GROUPS * 4 * NS    # 8192 gather descriptors
GELU_C = 0.044715
GELU_S = math.sqrt(2.0 / math.pi)


# ============================================================ host prep

def _np(x):
    return np.ascontiguousarray(np.asarray(x, dtype=np.float32))


def host_prep(inp):
    """inp: dict of full numpy inputs. Returns (in_maps, bo)."""
    Qf = _np(inp["Q_feature"])
    KVf = _np(inp["KV_feature"])
    wq = _np(inp["wq"]); bq = _np(inp["bq"])
    w_off_dw = _np(inp["w_off_dw"]); b_off_dw = _np(inp["b_off_dw"])
    ln_w = _np(inp["ln_w"]); ln_b = _np(inp["ln_b"])
    w_off_proj = _np(inp["w_off_proj"])
    wk = _np(inp["wk"]); wv = _np(inp["wv"]); bv = _np(inp["bv"])
    wo = _np(inp["wo"])

    # ---- gather source (per batch): rows (g, zy, xslot) of 64 floats:
    # [KV[c, z, y, x(slot)], KV[c, z, y, x(slot)+1]], x padded (-1..17).
    kvt_b = []
    for b in range(B):
        kv = KVf[b].reshape(GROUPS, GC, SP, SP, SP)
        kvp = np.zeros((GROUPS, GC, SP, SP, SP + 3), np.float32)
        kvp[..., 1:SP + 1] = kv
        a0 = kvp[..., 0:XSLOTS]
        a1 = kvp[..., 1:XSLOTS + 1]
        st = np.stack([a0, a1], axis=-1)          # [G, GC, Z, Y, XS, 2]
        st = st.transpose(0, 2, 3, 4, 5, 1)       # [G, Z, Y, XS, 2, GC]
        kvt_b.append(np.ascontiguousarray(st.reshape(GROUPS * G_ROWS, 2 * GC)))

    qf_b = [np.ascontiguousarray(Qf[b].reshape(CH, NQ)) for b in range(B)]

    # mean / bcast lhsT are block-structured (permutation-invariant)
    mean_lhsT = np.zeros((CH, GROUPS), np.float32)
    bcast_lhsT = np.zeros((GROUPS, CH), np.float32)
    for j in range(GROUPS):
        mean_lhsT[j * GC:(j + 1) * GC, j] = 1.0 / GC
        bcast_lhsT[j, j * GC:(j + 1) * GC] = 1.0

    # sampling reference grid (z, y, x), s = Z*64 + Y*8 + X
    r = (np.linspace(0.5, DK - 0.5, DK, dtype=np.float32) / DK) * 2 - 1
    zz, yy, xx = np.meshgrid(r, r, r, indexing="ij")
    axes = [zz.reshape(NS), yy.reshape(NS), xx.reshape(NS)]
    rxyz = np.zeros((12, NS), np.float32)
    for ax in range(3):
        for j in range(GROUPS):
            rxyz[ax * 4 + j] = (axes[ax] + 1.0) * 7.5 + 1.875

    wdw = w_off_dw.reshape(GC, KS, KS, KS)
    in_maps = []
    for c in range(HEADS):
        b, g = c // GROUPS, c % GROUPS
        # block j = group j (no permutation needed)
        border = list(range(GROUPS))
        # permuted channel list: block j holds group border[j]'s channels
        pch = np.concatenate([np.arange(gg * GC, (gg + 1) * GC)
                              for gg in border])

        wq_t = np.ascontiguousarray(wq[pch, :].T)       # [128 in, 128 out-perm]
        bq_c = bq[pch].reshape(CH, 1)

        convw = np.zeros((KS ** 3, CH, CH), np.float32)
        for dz in range(KS):
            for dy in range(KS):
                for dx in range(KS):
                    t = (dz * KS + dy) * KS + dx
                    d = np.tile(wdw[:, dz, dy, dx], GROUPS)  # per-channel,
                    # same for every group and thus permutation-invariant
                    convw[t][np.arange(CH), np.arange(CH)] = d
        bdw_c = np.tile(b_off_dw, GROUPS).reshape(CH, 1)
        lnw_c = np.tile(ln_w, GROUPS).reshape(CH, 1)
        lnb_c = np.tile(ln_b, GROUPS).reshape(CH, 1)

        projw_neg = np.zeros((CH, 12), np.float32)
        for j in range(GROUPS):
            for ax in range(3):
                projw_neg[j * GC:(j + 1) * GC, ax * 4 + j] = -w_off_proj[ax]

        goff = np.zeros((GROUPS, 1), np.float32)
        for j in range(GROUPS):
            goff[j] = 1.0 + border[j] * G_ROWS

        # attention-side weights; xs channel space is (block j, c) =
        # original channel pch
        hs = slice(g * GC, (g + 1) * GC)
        wk_h = [np.ascontiguousarray(
            (wk[g * GC + h * HC: g * GC + (h + 1) * HC, :][:, pch] * SCALE).T)
            for h in range(2)]
        wq_h = [np.ascontiguousarray(
            wq[g * GC + h * HC: g * GC + (h + 1) * HC, :].T)
            for h in range(2)]
        bq_h = [bq[g * GC + h * HC: g * GC + (h + 1) * HC].reshape(HC, 1)
                for h in range(2)]
        wv_t = np.ascontiguousarray(wv[hs, :][:, pch].T)     # [128, 32]

        wo_s = []
        for mh in range(2):
            m = np.zeros((CH, CH), np.float32)
            for h in range(2):
                j = h * 2 + mh
                cols = wo[:, g * GC + h * HC: g * GC + (h + 1) * HC]
                m[32 * j + 1: 32 * j + 17, :] = cols.T
            wo_s.append(np.ascontiguousarray(m))
        bc4 = np.zeros((GROUPS, CH), np.float32)
        for j in range(GROUPS):
            bc4[j, 32 * j + 1: 32 * j + 17] = 1.0
        ybias = (wo[:, hs] @ bv[hs]).reshape(CH, 1)

        in_maps.append({
            "qf": qf_b[b], "kvt": kvt_b[b],
            "wq_t": wq_t, "bq_c": bq_c,
            "convw": convw, "bdw_c": bdw_c,
            "lnw_c": lnw_c, "lnwn_c": -lnw_c, "lnb_c": lnb_c,
            "mean_lhsT": mean_lhsT, "bcast_lhsT": bcast_lhsT,
            "projw_neg": projw_neg, "rxyz": rxyz, "goff": goff,
            "wk_h0": wk_h[0], "wk_h1": wk_h[1], "wv_t": wv_t,
            "wq_h0": wq_h[0], "wq_h1": wq_h[1],
            "bq_h0": bq_h[0], "bq_h1": bq_h[1],
            "wo_sA": wo_s[0], "wo_sB": wo_s[1], "bc4": bc4,
            "ybias": ybias,
        })
    return in_maps


def host_post(results, bo):
    """results: list of 8 dicts with 'py' [128, 4096]."""
    bo = _np(bo)
    y = np.zeros((B, CH, NQ), np.float32)
    for c in range(HEADS):
        y[c // GROUPS] += results[c]["py"]
    y += bo.reshape(1, CH, 1)
    return y.reshape(B, CH, SP, SP, SP)


# ============================================================ device build

def build_program(tc: tile.TileContext, ctx):
    nc = tc.nc

    def dram_in(name, shape, dt=F32):
        return nc.dram_tensor(name, list(shape), dt, kind="ExternalInput").ap()

    qf = dram_in("qf", (CH, NQ))
    kvt = dram_in("kvt", (GROUPS * G_ROWS, 2 * GC))
    wq_t = dram_in("wq_t", (CH, CH))
    bq_c = dram_in("bq_c", (CH, 1))
    convw = dram_in("convw", (KS ** 3, CH, CH))
    bdw_c = dram_in("bdw_c", (CH, 1))
    lnw_c = dram_in("lnw_c", (CH, 1))
    lnwn_c = dram_in("lnwn_c", (CH, 1))
    lnb_c = dram_in("lnb_c", (CH, 1))
    mean_l = dram_in("mean_lhsT", (CH, GROUPS))
    bcast_l = dram_in("bcast_lhsT", (GROUPS, CH))
    projw = dram_in("projw_neg", (CH, 12))
    rxyz = dram_in("rxyz", (12, NS))
    goff = dram_in("goff", (GROUPS, 1))
    wq_h0 = dram_in("wq_h0", (CH, HC))
    wq_h1 = dram_in("wq_h1", (CH, HC))
    bq_h0 = dram_in("bq_h0", (HC, 1))
    bq_h1 = dram_in("bq_h1", (HC, 1))
    wk_h0 = dram_in("wk_h0", (CH, HC))
    wk_h1 = dram_in("wk_h1", (CH, HC))
    wv_t = dram_in("wv_t", (CH, GC))
    wo_sA = dram_in("wo_sA", (CH, CH))
    wo_sB = dram_in("wo_sB", (CH, CH))
    bc4 = dram_in("bc4", (GROUPS, CH))
    ybias = dram_in("ybias", (CH, 1))

    py = nc.dram_tensor("py", [CH, NQ], F32, kind="ExternalOutput").ap()

    idx_dram = nc.dram_tensor("idx_dram", [N_IDX], I16).ap()
    co_dram = nc.dram_tensor("co_dram", [12 * 3 * NS], F32).ap()
    w8_dram = nc.dram_tensor("w8_dram", [64 * 2 * CH], F32).ap()

    consts = ctx.enter_context(tc.tile_pool(name="consts", bufs=1))
    live = ctx.enter_context(tc.tile_pool(name="live", bufs=1))

    def load(ap, name, pool=consts, shape=None, dt=F32):
        t = pool.tile(list(shape or ap.shape), dt, tag=name, name=name)
        nc.sync.dma_start(t[:], ap)
        return t

    wq_sb = load(wq_t, "wq_sb")
    bq_sb = load(bq_c, "bq_sb")
    bdw_sb = load(bdw_c, "bdw_sb")
    lnw_sb = load(lnw_c, "lnw_sb")
    lnwn_sb = load(lnwn_c, "lnwn_sb")
    lnb_sb = load(lnb_c, "lnb_sb")
    mean_sb = load(mean_l, "mean_sb")
    bcast_sb = load(bcast_l, "bcast_sb")
    projw_sb = load(projw, "projw_sb")
    rxyz_sb = load(rxyz, "rxyz_sb")
    goff_sb = load(goff, "goff_sb")
    wq0_sb = load(wq_h0, "wq0_sb")
    wq1_sb = load(wq_h1, "wq1_sb")
    bq0_sb = load(bq_h0, "bq0_sb")
    bq1_sb = load(bq_h1, "bq1_sb")
    wk0_sb = load(wk_h0, "wk0_sb")
    wk1_sb = load(wk_h1, "wk1_sb")
    wv_sb = load(wv_t, "wv_sb")
    woA_sb = load(wo_sA, "woA_sb")
    woB_sb = load(wo_sB, "woB_sb")
    bc4_sb = load(bc4, "bc4_sb")
    ybias_sb = load(ybias, "ybias_sb")

    convw_sb = consts.tile([CH, KS ** 3 * CH], F32, tag="convw_sb", name="convw_sb")
    nc.sync.dma_start(convw_sb[:].rearrange("p (t c) -> p t c", t=KS ** 3),
                      convw.rearrange("t p c -> p t c"))

    ident = consts.tile([CH, CH], F32, tag="ident", name="ident")
    make_identity(nc, ident[:])

    # tiles that outlive the scratch phases
    q2_sb = live.tile([HC, 2 * NQ], F32, tag="q2_sb", name="q2_sb")
    k2_sb = live.tile([HC, 2 * NS], F32, tag="k2_sb", name="k2_sb")
    vt_sb = live.tile([CH, 4 * 64], F32, tag="vt_sb", name="vt_sb")
    xs_sb = live.tile([CH, GROUPS * CH], F32, tag="xs_sb", name="xs_sb")

    USE_F32R = False

    def r32(ap):
        return ap.bitcast(F32R) if USE_F32R else ap

    with tc.tile_pool(name="scratch", bufs=1) as scr, \
         tc.tile_pool(name="pq", bufs=2, space="PSUM") as pq_pool, \
         tc.tile_pool(name="pst", bufs=1, space="PSUM") as pst_pool:

        _cnt = [0]

        def st(tag, shape=(CH, NS), dt=F32):
            _cnt[0] += 1
            return scr.tile(list(shape), dt, tag=tag,
                            name=f"{tag}_{_cnt[0]}")

        # ---- phase 1: Q projection into a zero-padded 18^3 buffer ----
        qf_sb = st("tA", (CH, NQ))
        nc.sync.dma_start(qf_sb[:], qf)
        SPP = SP + 1
        q_pad = st("qpad", (CH, SPP ** 3))
        nc.gpsimd.memset(q_pad[:], 0.0)
        qp_zyx = q_pad[:].rearrange("p (z y x) -> p z y x", z=SPP, y=SPP)
        for i in range(NQ // 512):   # chunk = 2 z-slabs
            pq = pq_pool.tile([CH, 512], F32, tag="pq", name="pq")
            nc.tensor.matmul(pq[:], r32(wq_sb[:]),
                             r32(qf_sb[:, i * 512:(i + 1) * 512]),
                             start=True, stop=True)
            nc.scalar.activation(
                qp_zyx[:, 1 + 2 * i:3 + 2 * i, 1:SP + 1, 1:SP + 1],
                pq[:].rearrange("p (a b c) -> p a b c", a=2, b=SP),
                AF.Identity, bias=bq_sb[:])
        # per-head Q rows for attention (own group's two heads)
        for h, (wqh, bqh) in enumerate([(wq0_sb, bq0_sb), (wq1_sb, bq1_sb)]):
            for i in range(NQ // 512):
                pq2 = pst_pool.tile([HC, 512], F32, tag="pq2", name="pq2")
                nc.tensor.matmul(pq2[:], r32(wqh[:]),
                                 r32(qf_sb[:, i * 512:(i + 1) * 512]),
                                 start=True, stop=True)
                nc.vector.tensor_scalar(
                    q2_sb[:, h * NQ + i * 512:h * NQ + (i + 1) * 512],
                    pq2[:], bqh[:], None, ALU.add)

        # ---- phase 2: depthwise conv (stride 2) ----------------------
        pc = pst_pool.tile([CH, NS], F32, tag="psA", name="psA")
        first = True
        for dz in range(KS):
            for dy in range(KS):
                for dx in range(KS):
                    t = (dz * KS + dy) * KS + dx
                    rhs = qp_zyx[:, dz:dz + 2 * DK - 1:2,
                                 dy:dy + 2 * DK - 1:2,
                                 dx:dx + 2 * DK - 1:2]
                    nc.tensor.matmul(pc[:], r32(convw_sb[:, t * CH:(t + 1) * CH]),
                                     r32(rhs), start=first,
                                     stop=(t == KS ** 3 - 1))
                    first = False
        c_sb = st("s0")
        nc.scalar.activation(c_sb[:], pc[:], AF.Identity, bias=bdw_sb[:])

        # ---- phase 3: LayerNorm over 32-channel blocks ---------------
        csq = st("s1")
        nc.scalar.activation(csq[:], c_sb[:], AF.Square)
        pmu = pst_pool.tile([GROUPS, NS], F32, tag="psB", name="psB")
        nc.tensor.matmul(pmu[:], r32(mean_sb[:]), r32(c_sb[:]),
                         start=True, stop=True)
        pmsq = pst_pool.tile([GROUPS, NS], F32, tag="psC", name="psC")
        nc.tensor.matmul(pmsq[:], r32(mean_sb[:]), r32(csq[:]),
                         start=True, stop=True)
        mu2 = st("s2", (GROUPS, NS))
        nc.scalar.activation(mu2[:], pmu[:], AF.Square)
        var = st("s3", (GROUPS, NS))
        nc.vector.tensor_sub(var[:], pmsq[:], mu2[:])
        eps_sb = st("eps", (GROUPS, 1))
        nc.vector.memset(eps_sb[:], EPS)
        lnv = st("s2b", (GROUPS, NS))
        nc.scalar.activation(lnv[:], var[:], AF.Ln, bias=eps_sb[:])
        rstd = st("s1b", (GROUPS, NS))
        nc.scalar.activation(rstd[:], lnv[:], AF.Exp, scale=-0.5)
        murstd = st("s3b", (GROUPS, NS))
        nc.vector.tensor_mul(murstd[:], pmu[:], rstd[:])
        prb = pst_pool.tile([CH, NS], F32, tag="psB2", name="psB2")
        nc.tensor.matmul(prb[:], r32(bcast_sb[:]), r32(rstd[:]),
                         start=True, stop=True)
        pmb = pst_pool.tile([CH, NS], F32, tag="psC2", name="psC2")
        nc.tensor.matmul(pmb[:], r32(bcast_sb[:]), r32(murstd[:]),
                         start=True, stop=True)
        a_bc = st("s2")
        nc.vector.tensor_scalar(a_bc[:], prb[:], lnw_sb[:], None, ALU.mult)
        b_bc = st("s3")
        nc.vector.tensor_scalar(b_bc[:], pmb[:], lnwn_sb[:], lnb_sb[:],
                                ALU.mult, ALU.add)
        u = st("s4")
        nc.vector.tensor_mul(u[:], c_sb[:], a_bc[:])
        nc.vector.tensor_add(u[:], u[:], b_bc[:])

        # ---- phase 4: gelu (tanh approx; tanh via exp) ---------------
        usq = st("s0")
        nc.scalar.activation(usq[:], u[:], AF.Square)
        ucb = st("s1")
        nc.vector.tensor_mul(ucb[:], usq[:], u[:])
        g2 = st("s2")
        nc.vector.scalar_tensor_tensor(g2[:], ucb[:], GELU_C, u[:],
                                       ALU.mult, ALU.add)
        ge = st("s3")
        nc.scalar.activation(ge[:], g2[:], AF.Exp, scale=2.0 * GELU_S)
        nc.vector.tensor_scalar(ge[:], ge[:], 1.0, None, ALU.add)
        gr = st("s0")
        nc.vector.reciprocal(gr[:], ge[:])
        gneg = st("s1")
        nc.vector.scalar_tensor_tensor(gneg[:], gr[:], 1.0, u[:],
                                       ALU.subtract, ALU.mult)  # -gelu

        # ---- phase 5: offset proj + coords ---------------------------
        poff = pst_pool.tile([12, NS], F32, tag="psB", name="psB")
        nc.tensor.matmul(poff[:], r32(projw_sb[:]), r32(gneg[:]),
                         start=True, stop=True)
        ce = st("s2", (12, NS))
        nc.scalar.activation(ce[:], poff[:], AF.Exp, scale=2.0)
        nc.vector.tensor_scalar(ce[:], ce[:], 1.0, None, ALU.add)
        cr = st("s3", (12, NS))
        nc.vector.reciprocal(cr[:], ce[:])
        ixyz = st("s4", (12, NS))
        nc.vector.scalar_tensor_tensor(ixyz[:], cr[:], -3.75, rxyz_sb[:],
                                       ALU.mult, ALU.add)
        ci = st("s0", (12, NS), I32)
        nc.vector.tensor_copy(ci[:], ixyz[:])
        cf = st("s1", (12, NS))
        nc.vector.tensor_copy(cf[:], ci[:])
        fixm = st("s2", (12, NS))
        nc.vector.tensor_tensor(fixm[:], cf[:], ixyz[:], ALU.is_gt)
        f0 = st("s5", (12, NS))
        nc.vector.tensor_sub(f0[:], cf[:], fixm[:])
        tfrac = st("s3", (12, NS))
        nc.vector.tensor_sub(tfrac[:], ixyz[:], f0[:])
        m0 = st("s0", (12, NS))
        nc.vector.tensor_scalar(m0[:], f0[:], 0.0, None, ALU.is_ge)
        m1 = st("s1", (12, NS))
        nc.vector.tensor_scalar(m1[:], f0[:], 14.0, None, ALU.is_le)
        omt = st("s2", (12, NS))
        nc.vector.tensor_scalar(omt[:], tfrac[:], -1.0, 1.0, ALU.mult, ALU.add)

        big = st("big", (12, 3 * NS))
        nc.vector.tensor_copy(big[:, 0:NS], f0[:])
        nc.vector.tensor_mul(big[:, NS:2 * NS], omt[:], m0[:])
        nc.vector.tensor_mul(big[:, 2 * NS:3 * NS], tfrac[:], m1[:])
        nc.sync.dma_start(co_dram, big[:])
        co_g = st("co_g", (GROUPS, 9 * NS))
        nc.sync.dma_start(
            co_g[:].rearrange("g (ax k s) -> g ax k s", ax=3, k=3),
            co_dram.rearrange("(ax g k s) -> g ax k s", ax=3, g=4, k=3))

        def cgs(ax, kind):  # kind: 0 = floor, 1 = w0, 2 = w1
            o = (ax * 3 + kind) * NS
            return co_g[:, o:o + NS]

        zc0 = st("s0", (GROUPS, NS))
        zc1 = st("s1", (GROUPS, NS))
        yc0 = st("s2", (GROUPS, NS))
        yc1 = st("s3", (GROUPS, NS))
        nc.vector.tensor_scalar(zc0[:], cgs(0, 0), 0.0, 15.0, ALU.max, ALU.min)
        nc.vector.tensor_scalar(zc1[:], cgs(0, 0), 1.0, 0.0, ALU.add, ALU.max)
        nc.vector.tensor_scalar(zc1[:], zc1[:], 15.0, None, ALU.min)
        nc.vector.tensor_scalar(yc0[:], cgs(1, 0), 0.0, 15.0, ALU.max, ALU.min)
        nc.vector.tensor_scalar(yc1[:], cgs(1, 0), 1.0, 0.0, ALU.add, ALU.max)
        nc.vector.tensor_scalar(yc1[:], yc1[:], 15.0, None, ALU.min)
        xoff2 = st("s4", (GROUPS, NS))
        nc.vector.tensor_scalar(xoff2[:], cgs(2, 0), goff_sb[:], None, ALU.add)

        idxf = st("s5", (GROUPS, NS))
        idx16 = st("idx16", (GROUPS, 4 * NS), I16)
        wzy = st("wzy", (GROUPS, 4 * NS))
        zcs, ycs = [zc0, zc1], [yc0, yc1]
        for a in range(2):
            for bb in range(2):
                zy = a * 2 + bb
                nc.vector.scalar_tensor_tensor(
                    idxf[:], zcs[a][:], float(SP * XSLOTS), xoff2[:],
                    ALU.mult, ALU.add)
                nc.vector.scalar_tensor_tensor(
                    idxf[:], ycs[bb][:], float(XSLOTS), idxf[:],
                    ALU.mult, ALU.add)
                nc.vector.tensor_scalar(idxf[:], idxf[:], 0.0,
                                        float(GROUPS * G_ROWS - 1),
                                        ALU.max, ALU.min)
                nc.vector.tensor_copy(idx16[:, zy * NS:(zy + 1) * NS], idxf[:])
                nc.vector.tensor_mul(wzy[:, zy * NS:(zy + 1) * NS],
                                     cgs(0, 1 + a), cgs(1, 1 + bb))
        nc.sync.dma_start(idx_dram, idx16[:])
        # full trilinear corner weights w8[g, zy, x, s] = wzy * wx,
        # stored x-interleaved: [g, zy, s, x]
        w8s = st("w8s", (GROUPS, 4 * 2 * NS))
        w8sv = w8s[:].rearrange("g (zy s x) -> g zy s x", zy=4, x=2)
        for zy in range(4):
            for x in range(2):
                nc.vector.tensor_mul(w8sv[:, zy, :, x],
                                     wzy[:, zy * NS:(zy + 1) * NS],
                                     cgs(2, 1 + x))
        # bounce to DRAM with addressing (g zy si p x) = (j, p, x)
        w8d = w8_dram.rearrange("(g zy si p x) -> g zy si p x",
                                g=4, zy=4, si=4, x=2)
        for zy in range(4):
            nc.sync.dma_start(
                w8d[:, zy].rearrange("g si p x -> g (si p x)"),
                w8sv[:, zy].rearrange("g s x -> g (s x)"))

        # wrapped idx [128, 512]: global idx i at (i%16, i//16), x8 blocks
        import os as _os
        idxw = st("idxw", (CH, N_IDX // 16), I16)
        if _os.environ.get("DEFORM_NO_IDXW"):
            nc.vector.memset(idxw[:], 0)
        else:
            for rep in range(8):
                nc.gpsimd.dma_start(
                    idxw[rep * 16:(rep + 1) * 16, :],
                    idx_dram.rearrange("(col r) -> r col", r=16))

        # ---- phase 6: gather + trilinear combine ---------------------
        gth = scr.tile([CH, N_IDX // CH, 2 * GC], F32, tag="tA", name="tA")
        if _os.environ.get("DEFORM_NO_GATHER"):
            nc.vector.memset(gth[:], 0.25)
        else:
            NCHK = 32
            CH_I = N_IDX // NCHK          # 256 idx per gather call
            for k in range(NCHK):
                nc.gpsimd.dma_gather(
                    out_ap=gth[:, k * (CH_I // CH) * ...] if False else
                    gth[:, k * (CH_I // 128):(k + 1) * (CH_I // 128), :],
                    in_ap=kvt,
                    idxs_ap=idxw[:, k * (CH_I // 16):(k + 1) * (CH_I // 16)],
                    num_idxs=CH_I, num_idxs_reg=CH_I, elem_size=2 * GC)

        # stream order: i = ((g*4 + zy)*4 + si)*128 + p, sample s = si*128+p
        w8b = scr.tile([CH, 64, 2], F32, tag="tB", name="w8b")
        nc.sync.dma_start(
            w8b[:],
            w8_dram.rearrange("(j p x) -> p j x", j=64, x=2))
        t2 = scr.tile([CH, 64, 2 * GC], F32, tag="tC", name="t2")
        nc.vector.tensor_tensor(
            t2[:].rearrange("p j (x c) -> p j x c", x=2),
            gth[:].rearrange("p j (x c) -> p j x c", x=2),
            w8b[:].unsqueeze(3).broadcast_to([CH, 64, 2, GC]), ALU.mult)
        t2v = t2[:].rearrange("p (g zy si) e -> p g zy (si e)", g=4, zy=4)
        sa = st("sa", (CH, GROUPS, 4 * 2 * GC))
        sb = st("sb", (CH, GROUPS, 4 * 2 * GC))
        nc.vector.tensor_tensor(sa[:], t2v[:, :, 0], t2v[:, :, 1], ALU.add)
        nc.vector.tensor_tensor(sb[:], t2v[:, :, 2], t2v[:, :, 3], ALU.add)
        nc.vector.tensor_tensor(sa[:], sa[:], sb[:], ALU.add)
        sav = sa[:].rearrange("p g (si x c) -> p g si x c", si=4, x=2)
        xs_t = st("s0", (CH, 4, GROUPS, GC))   # [p, si, g, c]
        nc.vector.tensor_tensor(xs_t[:].rearrange("p si g c -> p g si c"),
                                sav[:, :, :, 0, :],
                                sav[:, :, :, 1, :], ALU.add)

        # ---- phase 7: transpose to xs [128 (blk,c), 512 n] -----------
        for si in range(4):
            pt = pst_pool.tile([CH, CH], F32, tag="psB", name="psB")
            nc.tensor.transpose(
                pt[:], xs_t[:, si].rearrange("p g c -> p (g c)"), ident[:])
            nc.scalar.activation(xs_sb[:, si * CH:(si + 1) * CH], pt[:],
                                 AF.Identity)

        # ---- phase 8: K and V-hat ------------------------------------
        for h, wkh in enumerate([wk0_sb, wk1_sb]):
            pk = pst_pool.tile([HC, NS], F32, tag="psC", name="psC")
            nc.tensor.matmul(pk[:], r32(wkh[:]), r32(xs_sb[:]),
                             start=True, stop=True)
            nc.scalar.activation(k2_sb[:, h * NS:(h + 1) * NS], pk[:],
                                 AF.Identity)
        nc.vector.memset(vt_sb[:], 0.0)
        nc.vector.memset(
            vt_sb[:].rearrange("p (n h s) -> p n h s", n=4, h=2)[:, :, :, 0:1],
            1.0)
        for nch in range(4):
            pv = pst_pool.tile([CH, GC], F32, tag="psA", name="psA")
            nc.tensor.matmul(pv[:], r32(xs_sb[:, nch * CH:(nch + 1) * CH]),
                             r32(wv_sb[:]), start=True, stop=True)
            nc.vector.tensor_copy(
                vt_sb[:].rearrange("p (n h s) -> p n h s", n=4, h=2)
                [:, nch, :, 1:17],
                pv[:].rearrange("p (h c) -> p h c", h=2))
        # (vt slot layout per n-chunk: [1 | V(16) | 0*15] x 2 heads, 64 wide)

    # ---- phase 9: attention loop -------------------------------------
    with tc.tile_pool(name="pA", bufs=2, space="PSUM") as pA, \
         tc.tile_pool(name="pO", bufs=2, space="PSUM") as pO, \
         tc.tile_pool(name="pR", bufs=1, space="PSUM") as pR, \
         tc.tile_pool(name="pY", bufs=1, space="PSUM") as pY, \
         tc.tile_pool(name="att_pool", bufs=3) as att_pool, \
         tc.tile_pool(name="opool", bufs=2) as opool:
        for mq in range(4):
            po = pO.tile([CH, 512], F32, tag="po", name="po")
            for h in range(2):
                for nch in range(4):
                    pa = pA.tile([CH, 1024], F32, tag="pa", name="pa")
                    for mh in range(2):
                        nc.tensor.matmul(
                            pa[:, mh * 512:(mh + 1) * 512],
                            r32(k2_sb[:, h * NS + nch * CH:
                                      h * NS + (nch + 1) * CH]),
                            r32(q2_sb[:, h * NQ + mq * 1024 + mh * 512:
                                      h * NQ + mq * 1024 + (mh + 1) * 512]),
                            start=True, stop=True)
                    att = att_pool.tile([CH, 1024], F32, tag="att", name="att")
                    nc.scalar.activation(att[:], pa[:], AF.Exp)
                    for mh in range(2):
                        j = h * 2 + mh
                        nc.tensor.matmul(
                            po[32 * j:32 * j + 32, :],
                            r32(vt_sb[:, nch * 64 + h * 32:
                                      nch * 64 + (h + 1) * 32]),
                            r32(att[:, mh * 512:(mh + 1) * 512]),
                            start=(nch == 0), stop=(nch == 3),
                            skip_group_check=True,
                            tile_position=(0, 32 * j))
            o_sb = opool.tile([CH, 512], F32, tag="o_sb", name="o_sb")
            nc.scalar.activation(o_sb[:], po[:], AF.Identity)
            den4 = opool.tile([GROUPS, 512], F32, tag="den4", name="den4")
            for j in range(4):
                nc.sync.dma_start(den4[j:j + 1, :], o_sb[32 * j:32 * j + 1, :])
            rd4 = opool.tile([GROUPS, 512], F32, tag="rd4", name="rd4")
            nc.vector.reciprocal(rd4[:], den4[:])
            prd = pR.tile([CH, 512], F32, tag="prd", name="prd")
            nc.tensor.matmul(prd[:], r32(bc4_sb[:]), r32(rd4[:]),
                             start=True, stop=True)
            on_sb = opool.tile([CH, 512], F32, tag="on_sb", name="on_sb")
            nc.vector.tensor_mul(on_sb[:], o_sb[:], prd[:])
            for mh, wos in enumerate([woA_sb, woB_sb]):
                pyp = pY.tile([CH, 512], F32, tag="pyp", name="pyp")
                nc.tensor.matmul(pyp[:], r32(wos[:]), r32(on_sb[:]),
                                 start=True, stop=True)
                y_sb = opool.tile([CH, 512], F32, tag="y_sb", name="y_sb")
                nc.scalar.activation(y_sb[:], pyp[:], AF.Identity,
                                     bias=ybias_sb[:])
                nc.sync.dma_start(
                    py[:, mq * 1024 + mh * 512:mq * 1024 + (mh + 1) * 512],
                    y_sb[:])


# ============================================================ entry points

_CACHE = {}


def _get_compiled():
    if "nc" in _CACHE:
        return _CACHE["nc"]
    from contextlib import ExitStack
    nc = bacc.Bacc("TRN2", target_bir_lowering=False, debug=False,
                   num_devices=HEADS)
    with tile.TileContext(nc) as tc:
        with ExitStack() as ctx:
            build_program(tc, ctx)
    nc.compile()
    _CACHE["nc"] = nc
    return nc


def _get_dispatch():
    """A cached PJRT dispatcher: same semantics as
    bass2jax.run_bass_via_pjrt (fresh host inputs in, numpy outputs back),
    but the jitted shard_map callable is built ONCE and the donated output
    buffers are created on-device instead of being shipped through the
    axon tunnel every call."""
    if "dispatch" in _CACHE:
        return _CACHE["dispatch"]
    import jax
    import jax.numpy as jnp
    from jax.sharding import Mesh, PartitionSpec, NamedSharding
    from jax.experimental.shard_map import shard_map
    from concourse.bass2jax import (_bass_exec_p, install_neuronx_cc_hook,
                                    partition_id_tensor)

    nc = _get_compiled()
    install_neuronx_cc_hook()
    n_cores = HEADS
    partition_name = (nc.partition_id_tensor.name
                      if nc.partition_id_tensor else None)
    in_names, out_names, out_avals = [], [], []
    for alloc in nc.m.functions[0].allocations:
        if not isinstance(alloc, mybir.MemoryLocationSet):
            continue
        name = alloc.memorylocations[0].name
        if alloc.kind == "ExternalInput":
            if name != partition_name:
                in_names.append(name)
        elif alloc.kind == "ExternalOutput":
            out_names.append(name)
            out_avals.append(jax.core.ShapedArray(
                tuple(alloc.tensor_shape), mybir.dt.np(alloc.dtype)))
    n_params = len(in_names)
    n_outs = len(out_avals)
    all_in_names = in_names + out_names
    if partition_name is not None:
        all_in_names.append(partition_name)

    def _body(*args):
        operands = list(args)
        if partition_name is not None:
            operands.append(partition_id_tensor())
        outs = _bass_exec_p.bind(
            *operands, out_avals=tuple(out_avals),
            in_names=tuple(all_in_names), out_names=tuple(out_names),
            lowering_input_output_aliases=(),
            sim_require_finite=True, sim_require_nnan=True, nc=nc)
        return tuple(outs)

    devices = jax.devices()[:n_cores]
    mesh = Mesh(np.asarray(devices), ("core",))
    in_specs = (PartitionSpec("core"),) * (n_params + n_outs)
    out_specs = (PartitionSpec("core"),) * n_outs
    donate = tuple(range(n_params, n_params + n_outs))
    sharded = jax.jit(
        shard_map(_body, mesh=mesh, in_specs=in_specs,
                  out_specs=out_specs, check_rep=False),
        donate_argnums=donate, keep_unused=True)

    shard = NamedSharding(mesh, PartitionSpec("core"))
    zmaker = jax.jit(
        lambda: tuple(
            jnp.zeros((n_cores * a.shape[0], *a.shape[1:]), a.dtype)
            for a in out_avals),
        out_shardings=(shard,) * n_outs)

    def dispatch(in_maps):
        concat_in = [
            np.concatenate([np.asarray(m[nm]) for m in in_maps], axis=0)
            for nm in in_names]
        out_arrs = sharded(*concat_in, *zmaker())
        return [
            {nm: np.asarray(out_arrs[i]).reshape(
                n_cores, *out_avals[i].shape)[c]
             for i, nm in enumerate(out_names)}
            for c in range(n_cores)]

    _CACHE["dispatch"] = dispatch
    return dispatch


def kernel(**inputs):
    dispatch = _get_dispatch()
    in_maps = host_prep(inputs)
    res = dispatch(in_maps)
    return host_post(res, inputs["bo"])


if __name__ == "__main__":
    _get_compiled()
    print("build + compile OK")



# revision 8
# speedup vs baseline: 8.1508x; 4.4026x over previous
"""Trainium2 Bass kernel for 3D deformable attention (8 NeuronCores).

Sharding: core c handles (b, mq) = (c // 4, c % 4): batch b, query
quarter mq (1024 of 4096 queries).  Each core runs the full offset /
sampling branch (all 4 groups, replicated within a batch), attention for
all 8 heads over its own 1024 queries, and the full output projection
y[:, mq-slice] = wo @ out + (wo @ bv + bo).  The host only concatenates
the 8 disjoint output slices — no summation, no bias.

All per-core data is packed into ONE f32 "blob" input [128, 4522]:
Q_feature[b] and KV_feature[b] bf16-packed (cols 0:2048 / 2048:4096),
the five [128,128] weight matrices bf16-packed (cols 4096:4416), and
small f32 constants after that.  The per-core query-quarter selection is
data-driven (a one-hot sel4 column in the blob), so a single SPMD NEFF
serves all 8 cores.

On-device builds (to minimize axon-tunnel upload bytes):
 - the x-interleaved trilinear gather table kvt [18432, 64] is built
   from the raw KV feature via 32 PE transposes + 258 strided DMAs
   (was a 4.7 MB host-precomputed ExternalInput);
 - the 27 depthwise-conv diagonal matrices are expanded from the raw
   [128, 27] taps with tensor_scalar against an identity (was 1.77 MB).

Numerical notes vs the jax reference:
 - bk is dropped: a per-(head,query) constant shift of attention logits
   is softmax-invariant.
 - bv enters as wo @ bv folded into the output bias (attention weights
   sum to 1 after normalization).
 - softmax skips the max-subtraction (logits are O(0.3)).
 - gelu(exact-erf) is replaced by the tanh approximation, with tanh and
   LayerNorm's rsqrt computed from exp/ln so one ACT table set serves
   the whole kernel.
 - inputs, the five big weight matrices, and the output are bf16.
"""

import math
import sys

for _p in ("/opt/trn_rl_repo",):
    if _p not in sys.path:
        sys.path.insert(0, _p)

import numpy as np
import ml_dtypes

import concourse.bass as bass
import concourse.mybir as mybir
import concourse.tile as tile
from concourse import bacc
from concourse.masks import make_identity

F32 = mybir.dt.float32
BF16 = mybir.dt.bfloat16
I32 = mybir.dt.int32
I16 = mybir.dt.int16
AF = mybir.ActivationFunctionType
ALU = mybir.AluOpType
NPBF16 = ml_dtypes.bfloat16

B = 2
CH = 128
HEADS = 8
GROUPS = 4
GC = CH // GROUPS     # 32
HC = CH // HEADS      # 16
SP = 16
NQ = SP * SP * SP     # 4096
QPC = NQ // 4         # 1024 queries per core
DK = 8
NS = DK * DK * DK     # 512 samples per group
KS = 3
EPS = 1e-5
SCALE = HC ** -0.5
XSLOTS = SP + 2       # x slots represent x = -1 .. 16 (18 slots)
G_ROWS = SP * SP * XSLOTS  # 4608 gather rows per group
N_IDX = GROUPS * 4 * NS    # 8192 gather descriptors
GELU_C = 0.044715
GELU_S = math.sqrt(2.0 / math.pi)
NCORES = 8

# ---- blob column layout (f32 columns) -------------------------------
O_QF = 0              # 2048 cols: Q_feature[b] bf16-packed [128, 4096]
O_KV = 2048           # 2048 cols: KV_feature[b] bf16-packed
O_W5 = 4096           # 5 * 64 cols: wq_t, wk_t, wv_t, wo_sA, wo_sB (bf16)
O_WDW = O_W5 + 5 * 64         # 4416: [128, 27] conv taps
O_BQ = O_WDW + 27             # 4443
O_BDW = O_BQ + 1              # 4444
O_LNW = O_BDW + 1             # 4445
O_LNWN = O_LNW + 1            # 4446
O_LNB = O_LNWN + 1            # 4447
O_YBO = O_LNB + 1             # 4448
O_PROJ = O_YBO + 1            # 4449: [128, 12]
O_SEL = O_PROJ + 12           # 4461: [128, 4] one-hot query-quarter
O_MEAN = O_SEL + 4            # 4465: [128, 4] mean lhsT
O_BCT = O_MEAN + 4            # 4469: [128, 4] bcast lhsT transposed
O_BC4 = O_BCT + 4             # 4473: [128, 4] denominator-bcast lhsT, transp
O_GOFF = O_BC4 + 4            # 4477: [4(rows), 1] group row offsets
O_RXYZ = O_GOFF + 1           # 4474: 48 cols, rxyz[r, q*128+p] at col r*4+q
W_BLOB = O_RXYZ + 48          # 4522
N_SM = W_BLOB - O_WDW         # 106 small f32 columns kept past scratch


# ============================================================ host prep

def _np(x):
    return np.ascontiguousarray(np.asarray(x, dtype=np.float32))


def _pack16(a):
    """[128, n] f32 -> bf16 -> view as [128, n//2] f32 container."""
    b16 = np.ascontiguousarray(a.astype(NPBF16))
    return b16.view(np.float32)


def host_prep(inp):
    """inp: dict of full numpy inputs. Returns in_maps (one blob per core)."""
    Qf = _np(inp["Q_feature"]).reshape(B, CH, NQ)
    KVf = _np(inp["KV_feature"]).reshape(B, CH, NQ)
    wq = _np(inp["wq"]); bq = _np(inp["bq"])
    w_off_dw = _np(inp["w_off_dw"]); b_off_dw = _np(inp["b_off_dw"])
    ln_w = _np(inp["ln_w"]); ln_b = _np(inp["ln_b"])
    w_off_proj = _np(inp["w_off_proj"])
    wk = _np(inp["wk"]); wv = _np(inp["wv"]); bv = _np(inp["bv"])
    wo = _np(inp["wo"]); bo = _np(inp["bo"])

    com = np.zeros((CH, W_BLOB - O_W5), np.float32)  # shared weight columns

    def put(off, arr):
        arr = np.asarray(arr, np.float32)
        com[:arr.shape[0], off - O_W5:off - O_W5 + arr.shape[1]] = arr

    put(O_W5 + 0 * 64, _pack16(wq.T))
    put(O_W5 + 1 * 64, _pack16((wk * SCALE).T))
    put(O_W5 + 2 * 64, _pack16(wv.T))
    wo_s = []
    for AB in range(2):
        m = np.zeros((CH, CH), np.float32)
        for h4 in range(4):
            h = AB * 4 + h4
            m[32 * h4 + 1: 32 * h4 + 17, :] = wo[:, HC * h: HC * (h + 1)].T
        wo_s.append(m)
    put(O_W5 + 3 * 64, _pack16(wo_s[0]))
    put(O_W5 + 4 * 64, _pack16(wo_s[1]))

    wdw = w_off_dw.reshape(GC, KS ** 3)
    put(O_WDW, np.tile(wdw, (GROUPS, 1)))          # [128, 27]
    put(O_BQ, bq.reshape(CH, 1))
    put(O_BDW, np.tile(b_off_dw, GROUPS).reshape(CH, 1))
    put(O_LNW, np.tile(ln_w, GROUPS).reshape(CH, 1))
    put(O_LNWN, -np.tile(ln_w, GROUPS).reshape(CH, 1))
    put(O_LNB, np.tile(ln_b, GROUPS).reshape(CH, 1))
    put(O_YBO, (wo @ bv + bo).reshape(CH, 1))

    projw_neg = np.zeros((CH, 12), np.float32)
    for j in range(GROUPS):
        for ax in range(3):
            projw_neg[j * GC:(j + 1) * GC, ax * 4 + j] = -w_off_proj[ax]
    put(O_PROJ, projw_neg)

    blk = np.zeros((CH, GROUPS), np.float32)       # block membership
    for j in range(GROUPS):
        blk[j * GC:(j + 1) * GC, j] = 1.0
    put(O_MEAN, blk / GC)
    put(O_BCT, blk)                                # bcast lhsT, transposed
    bc4 = np.zeros((CH, GROUPS), np.float32)
    for j in range(GROUPS):
        bc4[32 * j + 1:32 * j + 17, j] = 1.0
    put(O_BC4, bc4)                                # denom bcast, transposed

    goff = np.zeros((CH, 1), np.float32)
    for j in range(GROUPS):
        goff[j, 0] = 1.0 + j * G_ROWS
    put(O_GOFF, goff)

    # sampling reference grid (z, y, x): rxyz[ax*4+j, s] identical over j
    r = (np.linspace(0.5, DK - 0.5, DK, dtype=np.float32) / DK) * 2 - 1
    zz, yy, xx = np.meshgrid(r, r, r, indexing="ij")
    axes = [zz.reshape(NS), yy.reshape(NS), xx.reshape(NS)]
    rxyz = np.zeros((12, NS), np.float32)
    for ax in range(3):
        for j in range(GROUPS):
            rxyz[ax * 4 + j] = (axes[ax] + 1.0) * 7.5 + 1.875
    rpk = np.zeros((CH, 48), np.float32)
    for rr in range(12):
        for q in range(4):
            rpk[:, rr * 4 + q] = rxyz[rr, q * 128:(q + 1) * 128]
    put(O_RXYZ, rpk)

    in_maps = []
    for c in range(NCORES):
        b, mq = c // 4, c % 4
        blob = np.zeros((CH, W_BLOB), np.float32)
        blob[:, O_QF:O_QF + 2048] = _pack16(Qf[b])
        blob[:, O_KV:O_KV + 2048] = _pack16(KVf[b])
        blob[:, O_W5:] = com
        blob[:, O_SEL + mq] = 1.0
        in_maps.append({"blob": blob})
    return in_maps


def host_post(results, bo=None):
    """results: list of 8 dicts with 'py16' [128, 1024] bf16."""
    y = np.zeros((B, CH, NQ), np.float32)
    for c in range(NCORES):
        b, mq = c // 4, c % 4
        y[b][:, mq * QPC:(mq + 1) * QPC] = np.asarray(
            results[c]["py16"]).astype(np.float32)
    return y.reshape(B, CH, SP, SP, SP)


# ============================================================ device build

def build_program(tc: tile.TileContext, ctx):
    nc = tc.nc

    blob = nc.dram_tensor("blob", [CH, W_BLOB], F32, kind="ExternalInput").ap()
    py16 = nc.dram_tensor("py16", [CH, QPC], BF16, kind="ExternalOutput").ap()

    kvt_dram = nc.dram_tensor("kvt_dram", [GROUPS * G_ROWS, 2 * GC], F32).ap()
    idx_dram = nc.dram_tensor("idx_dram", [N_IDX], I16).ap()
    co_dram = nc.dram_tensor("co_dram", [12 * 3 * NS], F32).ap()
    w8_dram = nc.dram_tensor("w8_dram", [64 * 2 * CH], F32).ap()

    consts = ctx.enter_context(tc.tile_pool(name="consts", bufs=1))
    live = ctx.enter_context(tc.tile_pool(name="live", bufs=1))

    ident = consts.tile([CH, CH], F32, tag="ident", name="ident")
    make_identity(nc, ident[:])

    # small f32 constants that must outlive the scratch phase
    smalls = consts.tile([CH, N_SM], F32, tag="smalls", name="smalls")

    def bcol(off, n=1):
        return smalls[:, off - O_WDW:off - O_WDW + n]

    # [row-dim < 128] constants come via rearranged DMA straight from DRAM
    bcast_sb = consts.tile([GROUPS, CH], F32, tag="bcast_sb", name="bcast_sb")
    nc.sync.dma_start(bcast_sb[:],
                      blob[:, O_BCT:O_BCT + 4].rearrange("p j -> j p"))
    rxyz_sb = consts.tile([12, NS], F32, tag="rxyz_sb", name="rxyz_sb")
    nc.sync.dma_start(
        rxyz_sb[:].rearrange("r (q p) -> r q p", q=4),
        blob[:, O_RXYZ:O_RXYZ + 48].rearrange("p (r q) -> r q p", r=12))
    goff_sb = consts.tile([GROUPS, 1], F32, tag="goff_sb", name="goff_sb")
    nc.sync.dma_start(goff_sb[:], blob[0:GROUPS, O_GOFF:O_GOFF + 1])
    bq8 = consts.tile([HC, HEADS], F32, tag="bq8", name="bq8")
    nc.sync.dma_start(
        bq8[:], blob[:, O_BQ:O_BQ + 1].rearrange("(h c) j -> c (h j)", h=8))

    # bc4 lhsT for denominator broadcast: rows j, cols 32j+1..32j+17 = 1
    bc4_sb = consts.tile([GROUPS, CH], F32, tag="bc4_sb", name="bc4_sb")
    nc.sync.dma_start(bc4_sb[:],
                      blob[:, O_BC4:O_BC4 + 4].rearrange("p j -> j p"))

    # five bf16-packed [128,128] matrices -> f32 SBUF tiles
    w5 = consts.tile([CH, 5 * CH], F32, tag="w5", name="w5")
    wq_sb = w5[:, 0 * CH:1 * CH]
    wk_sb = w5[:, 1 * CH:2 * CH]
    wv_sb = w5[:, 2 * CH:3 * CH]
    woA_sb = w5[:, 3 * CH:4 * CH]
    woB_sb = w5[:, 4 * CH:5 * CH]

    convw_sb = consts.tile([CH, KS ** 3 * CH], F32, tag="convw_sb",
                           name="convw_sb")

    # tiles that outlive the scratch phases
    qsl = live.tile([CH, QPC], F32, tag="qsl", name="qsl")
    vt_sb = live.tile([CH, 4 * 2 * 4 * GC], F32, tag="vt_sb", name="vt_sb")
    xs_sb = live.tile([CH, GROUPS * CH], F32, tag="xs_sb", name="xs_sb")

    with tc.tile_pool(name="scratch", bufs=1) as scr, \
         tc.tile_pool(name="ps", bufs=2, space="PSUM") as ps_pool, \
         tc.tile_pool(name="pst", bufs=1, space="PSUM") as pst_pool:

        _cnt = [0]

        def st(tag, shape=(CH, NS), dt=F32):
            _cnt[0] += 1
            return scr.tile(list(shape), dt, tag=tag, name=f"{tag}_{_cnt[0]}")

        def ps512():
            return ps_pool.tile([CH, 512], F32, tag="ps512", name="ps512")

        # ---- phase 0: load blob, unpack, build conv diagonals --------
        blob_sb = st("blob", (CH, W_BLOB))
        nc.sync.dma_start(blob_sb[:], blob)
        nc.vector.tensor_copy(smalls[:], blob_sb[:, O_WDW:W_BLOB])
        nc.scalar.activation(
            w5[:], blob_sb[:, O_W5:O_W5 + 5 * 64].bitcast(BF16), AF.Identity)
        for t in range(KS ** 3):
            nc.vector.tensor_scalar(convw_sb[:, t * CH:(t + 1) * CH],
                                    ident[:], bcol(O_WDW + t), None, ALU.mult)

        qf_sb = st("qf", (CH, NQ))
        nc.scalar.activation(qf_sb[:], blob_sb[:, O_QF:O_QF + 2048]
                             .bitcast(BF16), AF.Identity)
        kv_sb = st("kv", (CH, NQ))
        nc.scalar.activation(kv_sb[:], blob_sb[:, O_KV:O_KV + 2048]
                             .bitcast(BF16), AF.Identity)

        # transpose kv to spatial-major: tile k holds [sp 128k.., ch]
        kvT_sb = st("kvT", (CH, NQ))
        for k in range(NQ // CH):
            pt = ps512()
            nc.tensor.transpose(pt[:, 0:CH], kv_sb[:, k * CH:(k + 1) * CH],
                                ident[:])
            nc.scalar.activation(kvT_sb[:, k * CH:(k + 1) * CH], pt[:, 0:CH],
                                 AF.Identity)

        # scatter into the x-interleaved gather table
        # row (g, z, y, xs) = [KV[g,:,z,y,xs-1], KV[g,:,z,y,xs]]
        kvt_v = kvt_dram.rearrange("(g z y xs) w -> g z y xs w",
                                   g=GROUPS, z=SP, y=SP)
        for k in range(NQ // CH):
            z, yh = k // 2, k % 2
            for g in range(GROUPS):
                src = kvT_sb[:, k * CH + g * GC:k * CH + (g + 1) * GC]
                nc.sync.dma_start(
                    kvt_v[g, z, yh * 8:yh * 8 + 8, 1:SP + 1, 0:GC], src)
                nc.sync.dma_start(
                    kvt_v[g, z, yh * 8:yh * 8 + 8, 0:SP, GC:2 * GC], src)
        # finite-fill the two fetched-but-masked edge columns
        zfill = st("zfill", (CH, 256))
        nc.vector.memset(zfill[:], 0.0)
        nc.sync.dma_start(
            kvt_v[:, :, :, 0, 0:GC].rearrange("g z y c -> (g z y) c")
            .rearrange("(a b) c -> a b c", a=CH),
            zfill[:].rearrange("p (b c) -> p b c", c=GC))
        nc.sync.dma_start(
            kvt_v[:, :, :, SP, GC:2 * GC].rearrange("g z y c -> (g z y) c")
            .rearrange("(a b) c -> a b c", a=CH),
            zfill[:].rearrange("p (b c) -> p b c", c=GC))

        # ---- phase 1: Q projection -----------------------------------
        # full projection into a zero-padded 17^3 buffer (conv input)
        SPP = SP + 1
        q_pad = st("qpad", (CH, SPP ** 3))
        nc.gpsimd.memset(q_pad[:], 0.0)
        qp_zyx = q_pad[:].rearrange("p (z y x) -> p z y x", z=SPP, y=SPP)
        for i in range(NQ // 512):   # chunk = 2 z-slabs
            pq = ps512()
            nc.tensor.matmul(pq[:], wq_sb,
                             qf_sb[:, i * 512:(i + 1) * 512],
                             start=True, stop=True)
            nc.scalar.activation(
                qp_zyx[:, 1 + 2 * i:3 + 2 * i, 1:SP + 1, 1:SP + 1],
                pq[:].rearrange("p (a b c) -> p a b c", a=2, b=SP),
                AF.Identity, bias=bcol(O_BQ))

        # select this core's 1024 queries:  qsl = sum_m sel[m] * qf[:, m]
        tmp_q = st("tmpq", (CH, QPC))
        nc.vector.tensor_scalar(qsl[:], qf_sb[:, 0:QPC],
                                bcol(O_SEL + 0), None, ALU.mult)
        for m in range(1, 4):
            nc.vector.tensor_scalar(tmp_q[:], qf_sb[:, m * QPC:(m + 1) * QPC],
                                    bcol(O_SEL + m), None, ALU.mult)
            nc.vector.tensor_add(qsl[:], qsl[:], tmp_q[:])

        # ---- phase 2: depthwise conv (stride 2) ----------------------
        pc = pst_pool.tile([CH, NS], F32, tag="psA", name="psA")
        first = True
        for dz in range(KS):
            for dy in range(KS):
                for dx in range(KS):
                    t = (dz * KS + dy) * KS + dx
                    rhs = qp_zyx[:, dz:dz + 2 * DK - 1:2,
                                 dy:dy + 2 * DK - 1:2,
                                 dx:dx + 2 * DK - 1:2]
                    nc.tensor.matmul(pc[:], convw_sb[:, t * CH:(t + 1) * CH],
                                     rhs, start=first,
                                     stop=(t == KS ** 3 - 1))
                    first = False
        c_sb = st("s0")
        nc.scalar.activation(c_sb[:], pc[:], AF.Identity, bias=bcol(O_BDW))

        # ---- phase 3: LayerNorm over 32-channel blocks ---------------
        csq = st("s1")
        nc.scalar.activation(csq[:], c_sb[:], AF.Square)
        pmu = pst_pool.tile([GROUPS, NS], F32, tag="psB", name="psB")
        nc.tensor.matmul(pmu[:], bcol(O_MEAN, 4), c_sb[:],
                         start=True, stop=True)
        pmsq = pst_pool.tile([GROUPS, NS], F32, tag="psC", name="psC")
        nc.tensor.matmul(pmsq[:], bcol(O_MEAN, 4), csq[:],
                         start=True, stop=True)
        mu2 = st("s2", (GROUPS, NS))
        nc.scalar.activation(mu2[:], pmu[:], AF.Square)
        var = st("s3", (GROUPS, NS))
        nc.vector.tensor_sub(var[:], pmsq[:], mu2[:])
        eps_sb = st("eps", (GROUPS, 1))
        nc.vector.memset(eps_sb[:], EPS)
        lnv = st("s2b", (GROUPS, NS))
        nc.scalar.activation(lnv[:], var[:], AF.Ln, bias=eps_sb[:])
        rstd = st("s1b", (GROUPS, NS))
        nc.scalar.activation(rstd[:], lnv[:], AF.Exp, scale=-0.5)
        murstd = st("s3b", (GROUPS, NS))
        nc.vector.tensor_mul(murstd[:], pmu[:], rstd[:])
        prb = pst_pool.tile([CH, NS], F32, tag="psA", name="psA2")
        nc.tensor.matmul(prb[:], bcast_sb[:], rstd[:], start=True, stop=True)
        pmb = pst_pool.tile([CH, NS], F32, tag="psA", name="psA3")
        nc.tensor.matmul(pmb[:], bcast_sb[:], murstd[:], start=True, stop=True)
        a_bc = st("s2")
        nc.vector.tensor_scalar(a_bc[:], prb[:], bcol(O_LNW), None, ALU.mult)
        b_bc = st("s3")
        nc.vector.tensor_scalar(b_bc[:], pmb[:], bcol(O_LNWN), bcol(O_LNB),
                                ALU.mult, ALU.add)
        u = st("s4")
        nc.vector.tensor_mul(u[:], c_sb[:], a_bc[:])
        nc.vector.tensor_add(u[:], u[:], b_bc[:])

        # ---- phase 4: gelu (tanh approx; tanh via exp) ---------------
        usq = st("s0")
        nc.scalar.activation(usq[:], u[:], AF.Square)
        ucb = st("s1")
        nc.vector.tensor_mul(ucb[:], usq[:], u[:])
        g2 = st("s2")
        nc.vector.scalar_tensor_tensor(g2[:], ucb[:], GELU_C, u[:],
                                       ALU.mult, ALU.add)
        ge = st("s3")
        nc.scalar.activation(ge[:], g2[:], AF.Exp, scale=2.0 * GELU_S)
        nc.vector.tensor_scalar(ge[:], ge[:], 1.0, None, ALU.add)
        gr = st("s0")
        nc.vector.reciprocal(gr[:], ge[:])
        gneg = st("s1")
        nc.vector.scalar_tensor_tensor(gneg[:], gr[:], 1.0, u[:],
                                       ALU.subtract, ALU.mult)  # -gelu

        # ---- phase 5: offset proj + coords ---------------------------
        poff = pst_pool.tile([12, NS], F32, tag="psB", name="psB2")
        nc.tensor.matmul(poff[:], bcol(O_PROJ, 12), gneg[:],
                         start=True, stop=True)
        ce = st("s2", (12, NS))
        nc.scalar.activation(ce[:], poff[:], AF.Exp, scale=2.0)
        nc.vector.tensor_scalar(ce[:], ce[:], 1.0, None, ALU.add)
        cr = st("s3", (12, NS))
        nc.vector.reciprocal(cr[:], ce[:])
        ixyz = st("s4", (12, NS))
        nc.vector.scalar_tensor_tensor(ixyz[:], cr[:], -3.75, rxyz_sb[:],
                                       ALU.mult, ALU.add)
        ci = st("s0", (12, NS), I32)
        nc.vector.tensor_copy(ci[:], ixyz[:])
        cf = st("s1", (12, NS))
        nc.vector.tensor_copy(cf[:], ci[:])
        fixm = st("s2", (12, NS))
        nc.vector.tensor_tensor(fixm[:], cf[:], ixyz[:], ALU.is_gt)
        f0 = st("s5", (12, NS))
        nc.vector.tensor_sub(f0[:], cf[:], fixm[:])
        tfrac = st("s3", (12, NS))
        nc.vector.tensor_sub(tfrac[:], ixyz[:], f0[:])
        m0 = st("s0", (12, NS))
        nc.vector.tensor_scalar(m0[:], f0[:], 0.0, None, ALU.is_ge)
        m1 = st("s1", (12, NS))
        nc.vector.tensor_scalar(m1[:], f0[:], 14.0, None, ALU.is_le)
        omt = st("s2", (12, NS))
        nc.vector.tensor_scalar(omt[:], tfrac[:], -1.0, 1.0, ALU.mult, ALU.add)

        big = st("big", (12, 3 * NS))
        nc.vector.tensor_copy(big[:, 0:NS], f0[:])
        nc.vector.tensor_mul(big[:, NS:2 * NS], omt[:], m0[:])
        nc.vector.tensor_mul(big[:, 2 * NS:3 * NS], tfrac[:], m1[:])
        nc.sync.dma_start(co_dram, big[:])
        co_g = st("co_g", (GROUPS, 9 * NS))
        nc.sync.dma_start(
            co_g[:].rearrange("g (ax k s) -> g ax k s", ax=3, k=3),
            co_dram.rearrange("(ax g k s) -> g ax k s", ax=3, g=4, k=3))

        def cgs(ax, kind):  # kind: 0 = floor, 1 = w0, 2 = w1
            o = (ax * 3 + kind) * NS
            return co_g[:, o:o + NS]

        zc0 = st("s0", (GROUPS, NS))
        zc1 = st("s1", (GROUPS, NS))
        yc0 = st("s2", (GROUPS, NS))
        yc1 = st("s3", (GROUPS, NS))
        nc.vector.tensor_scalar(zc0[:], cgs(0, 0), 0.0, 15.0, ALU.max, ALU.min)
        nc.vector.tensor_scalar(zc1[:], cgs(0, 0), 1.0, 0.0, ALU.add, ALU.max)
        nc.vector.tensor_scalar(zc1[:], zc1[:], 15.0, None, ALU.min)
        nc.vector.tensor_scalar(yc0[:], cgs(1, 0), 0.0, 15.0, ALU.max, ALU.min)
        nc.vector.tensor_scalar(yc1[:], cgs(1, 0), 1.0, 0.0, ALU.add, ALU.max)
        nc.vector.tensor_scalar(yc1[:], yc1[:], 15.0, None, ALU.min)
        xoff2 = st("s4", (GROUPS, NS))
        nc.vector.tensor_scalar(xoff2[:], cgs(2, 0), goff_sb[:], None, ALU.add)

        idxf = st("s5", (GROUPS, NS))
        idx16 = st("idx16", (GROUPS, 4 * NS), I16)
        wzy = st("wzy", (GROUPS, 4 * NS))
        zcs, ycs = [zc0, zc1], [yc0, yc1]
        for a in range(2):
            for bb in range(2):
                zy = a * 2 + bb
                nc.vector.scalar_tensor_tensor(
                    idxf[:], zcs[a][:], float(SP * XSLOTS), xoff2[:],
                    ALU.mult, ALU.add)
                nc.vector.scalar_tensor_tensor(
                    idxf[:], ycs[bb][:], float(XSLOTS), idxf[:],
                    ALU.mult, ALU.add)
                nc.vector.tensor_scalar(idxf[:], idxf[:], 0.0,
                                        float(GROUPS * G_ROWS - 1),
                                        ALU.max, ALU.min)
                nc.vector.tensor_copy(idx16[:, zy * NS:(zy + 1) * NS], idxf[:])
                nc.vector.tensor_mul(wzy[:, zy * NS:(zy + 1) * NS],
                                     cgs(0, 1 + a), cgs(1, 1 + bb))
        nc.sync.dma_start(idx_dram, idx16[:])
        # full trilinear corner weights w8[g, zy, x, s] = wzy * wx,
        # streamed piecewise to DRAM addressed (g zy si p x)
        w8d = w8_dram.rearrange("(g zy si p x) -> g zy si p x",
                                g=4, zy=4, si=4, x=2)
        for zy in range(4):
            for x in range(2):
                w8p = st("w8p", (GROUPS, NS))
                nc.vector.tensor_mul(w8p[:],
                                     wzy[:, zy * NS:(zy + 1) * NS],
                                     cgs(2, 1 + x))
                nc.sync.dma_start(
                    w8d[:, zy, :, :, x],
                    w8p[:].rearrange("g (si p) -> g si p", si=4))

        # wrapped idx [128, 512]: global idx i at (i%16, i//16), x8 blocks
        idxw = st("idxw", (CH, N_IDX // 16), I16)
        for rep in range(8):
            nc.gpsimd.dma_start(
                idxw[rep * 16:(rep + 1) * 16, :],
                idx_dram.rearrange("(col r) -> r col", r=16))

        # ---- phase 6: gather + trilinear combine ---------------------
        gth = scr.tile([CH, N_IDX // CH, 2 * GC], F32, tag="gth", name="gth")
        NCHK = 32
        CH_I = N_IDX // NCHK          # 256 idx per gather call
        for k in range(NCHK):
            nc.gpsimd.dma_gather(
                out_ap=gth[:, k * (CH_I // 128):(k + 1) * (CH_I // 128), :],
                in_ap=kvt_dram,
                idxs_ap=idxw[:, k * (CH_I // 16):(k + 1) * (CH_I // 16)],
                num_idxs=CH_I, num_idxs_reg=CH_I, elem_size=2 * GC)

        # stream order: i = ((g*4 + zy)*4 + si)*128 + p, sample s = si*128+p
        w8b = scr.tile([CH, 64, 2], F32, tag="tB", name="w8b")
        nc.sync.dma_start(
            w8b[:],
            w8_dram.rearrange("(j p x) -> p j x", j=64, x=2))
        nc.vector.tensor_tensor(
            gth[:].rearrange("p j (x c) -> p j x c", x=2),
            gth[:].rearrange("p j (x c) -> p j x c", x=2),
            w8b[:].unsqueeze(3).broadcast_to([CH, 64, 2, GC]), ALU.mult)
        t2v = gth[:].rearrange("p (g zy si) e -> p g zy (si e)", g=4, zy=4)
        sa = st("sa", (CH, GROUPS, 4 * 2 * GC))
        sb = st("sb", (CH, GROUPS, 4 * 2 * GC))
        nc.vector.tensor_tensor(sa[:], t2v[:, :, 0], t2v[:, :, 1], ALU.add)
        nc.vector.tensor_tensor(sb[:], t2v[:, :, 2], t2v[:, :, 3], ALU.add)
        nc.vector.tensor_tensor(sa[:], sa[:], sb[:], ALU.add)
        sav = sa[:].rearrange("p g (si x c) -> p g si x c", si=4, x=2)
        xs_t = st("s0", (CH, 4, GROUPS, GC))   # [p, si, g, c]
        nc.vector.tensor_tensor(xs_t[:].rearrange("p si g c -> p g si c"),
                                sav[:, :, :, 0, :],
                                sav[:, :, :, 1, :], ALU.add)

        # ---- phase 7: transpose to xs [128 (g,c), 512 n] -------------
        for si in range(4):
            pt = ps512()
            nc.tensor.transpose(
                pt[:, 0:CH], xs_t[:, si].rearrange("p g c -> p (g c)"),
                ident[:])
            nc.scalar.activation(xs_sb[:, si * CH:(si + 1) * CH], pt[:, 0:CH],
                                 AF.Identity)

        # ---- phase 8: V-hat ------------------------------------------
        nc.vector.memset(vt_sb[:], 0.0)
        nc.vector.memset(
            vt_sb[:].rearrange("p (n h s) -> p n h s", n=4, h=HEADS)
            [:, :, :, 0:1], 1.0)
        for nch in range(4):
            pv = ps512()
            nc.tensor.matmul(pv[:, 0:CH],
                             xs_sb[:, nch * CH:(nch + 1) * CH],
                             wv_sb, start=True, stop=True)
            nc.vector.tensor_copy(
                vt_sb[:].rearrange("p (n h s) -> p n h s", n=4, h=HEADS)
                [:, nch, :, 1:HC + 1],
                pv[:, 0:CH].rearrange("p (h c) -> p h c", h=HEADS))
        # (vt layout per n-chunk: 8 x [1 | V(16) | 0*15], 256 wide)

    # ---- phase 9: attention loop -------------------------------------
    # per query-half: Q/K per head on the fly, logits -> exp -> V-hat
    # accumulation (denominator in row 0 of each 32-block), normalize,
    # project through wo and emit bf16.
    with tc.tile_pool(name="pA", bufs=2, space="PSUM") as pA, \
         tc.tile_pool(name="pO", bufs=2, space="PSUM") as pO, \
         tc.tile_pool(name="pR", bufs=1, space="PSUM") as pR, \
         tc.tile_pool(name="pY", bufs=1, space="PSUM") as pY, \
         tc.tile_pool(name="att_pool", bufs=3) as att_pool, \
         tc.tile_pool(name="opool", bufs=2) as opool:
        for qh in range(2):
            on_tiles = {}
            for AB in range(2):
                po = pO.tile([CH, 512], F32, tag="po", name="po")
                for h4 in range(4):
                    h = AB * 4 + h4
                    pq2 = pA.tile([HC, 512], F32, tag="p16", name="p16q")
                    nc.tensor.matmul(pq2[:], wq_sb[:, HC * h:HC * (h + 1)],
                                     qsl[:, qh * 512:(qh + 1) * 512],
                                     start=True, stop=True)
                    q2h = att_pool.tile([HC, 512], F32, tag="q2h", name="q2h")
                    nc.vector.tensor_scalar(q2h[:], pq2[:], bq8[:, h:h + 1],
                                            None, ALU.add)
                    pk = pA.tile([HC, 512], F32, tag="p16", name="p16k")
                    nc.tensor.matmul(pk[:], wk_sb[:, HC * h:HC * (h + 1)],
                                     xs_sb[:], start=True, stop=True)
                    k2h = att_pool.tile([HC, 512], F32, tag="k2h", name="k2h")
                    nc.scalar.activation(k2h[:], pk[:], AF.Identity)
                    for nch in range(4):
                        pa = pA.tile([CH, 512], F32, tag="pa", name="pa")
                        nc.tensor.matmul(
                            pa[:], k2h[:, nch * CH:(nch + 1) * CH], q2h[:],
                            start=True, stop=True)
                        att = att_pool.tile([CH, 512], F32, tag="att",
                                            name="att")
                        nc.scalar.activation(att[:], pa[:], AF.Exp)
                        nc.tensor.matmul(
                            po[32 * h4:32 * h4 + 32, :],
                            vt_sb[:, nch * 256 + h * 32:
                                  nch * 256 + h * 32 + 32],
                            att[:], start=(nch == 0), stop=(nch == 3),
                            skip_group_check=True,
                            tile_position=(0, 32 * h4))
                o_sb = opool.tile([CH, 512], F32, tag="o_sb", name="o_sb")
                nc.scalar.activation(o_sb[:], po[:], AF.Identity)
                den4 = opool.tile([GROUPS, 512], F32, tag="den4", name="den4")
                for j in range(4):
                    nc.sync.dma_start(den4[j:j + 1, :],
                                      o_sb[32 * j:32 * j + 1, :])
                rd4 = opool.tile([GROUPS, 512], F32, tag="rd4", name="rd4")
                nc.vector.reciprocal(rd4[:], den4[:])
                prd = pR.tile([CH, 512], F32, tag="prd", name="prd")
                nc.tensor.matmul(prd[:], bc4_sb[:], rd4[:],
                                 start=True, stop=True)
                on_sb = opool.tile([CH, 512], F32, tag=f"on{AB}",
                                   name=f"on{AB}")
                nc.vector.tensor_mul(on_sb[:], o_sb[:], prd[:])
                on_tiles[AB] = on_sb

            pyp = pY.tile([CH, 512], F32, tag="pyp", name="pyp")
            for AB in range(2):
                nc.tensor.matmul(pyp[:],
                                 (woA_sb if AB == 0 else woB_sb),
                                 on_tiles[AB][:],
                                 start=(AB == 0), stop=(AB == 1))
            y16 = opool.tile([CH, 512], BF16, tag="y16", name="y16")
            nc.scalar.activation(y16[:], pyp[:], AF.Identity,
                                 bias=bcol(O_YBO))
            nc.sync.dma_start(py16[:, qh * 512:(qh + 1) * 512], y16[:])


# ============================================================ entry points

_CACHE = {}


def _get_compiled():
    if "nc" in _CACHE:
        return _CACHE["nc"]
    from contextlib import ExitStack
    nc = bacc.Bacc("TRN2", target_bir_lowering=False, debug=False,
                   num_devices=NCORES)
    with tile.TileContext(nc) as tc:
        with ExitStack() as ctx:
            build_program(tc, ctx)
    nc.compile()
    _CACHE["nc"] = nc
    return nc


def _get_dispatch():
    """A cached PJRT dispatcher: same semantics as
    bass2jax.run_bass_via_pjrt (fresh host inputs in, numpy outputs back),
    but the jitted shard_map callable is built ONCE and the donated output
    buffers are created on-device instead of being shipped through the
    axon tunnel every call."""
    if "dispatch" in _CACHE:
        return _CACHE["dispatch"]
    import jax
    import jax.numpy as jnp
    from jax.sharding import Mesh, PartitionSpec, NamedSharding
    from jax.experimental.shard_map import shard_map
    from concourse.bass2jax import (_bass_exec_p, install_neuronx_cc_hook,
                                    partition_id_tensor)

    nc = _get_compiled()
    install_neuronx_cc_hook()
    n_cores = NCORES
    partition_name = (nc.partition_id_tensor.name
                      if nc.partition_id_tensor else None)
    in_names, out_names, out_avals = [], [], []
    for alloc in nc.m.functions[0].allocations:
        if not isinstance(alloc, mybir.MemoryLocationSet):
            continue
        name = alloc.memorylocations[0].name
        if alloc.kind == "ExternalInput":
            if name != partition_name:
                in_names.append(name)
        elif alloc.kind == "ExternalOutput":
            out_names.append(name)
            out_avals.append(jax.core.ShapedArray(
                tuple(alloc.tensor_shape), mybir.dt.np(alloc.dtype)))
    n_params = len(in_names)
    n_outs = len(out_avals)
    all_in_names = in_names + out_names
    if partition_name is not None:
        all_in_names.append(partition_name)

    def _body(*args):
        operands = list(args)
        if partition_name is not None:
            operands.append(partition_id_tensor())
        outs = _bass_exec_p.bind(
            *operands, out_avals=tuple(out_avals),
            in_names=tuple(all_in_names), out_names=tuple(out_names),
            lowering_input_output_aliases=(),
            sim_require_finite=True, sim_require_nnan=True, nc=nc)
        return tuple(outs)

    devices = jax.devices()[:n_cores]
    mesh = Mesh(np.asarray(devices), ("core",))
    in_specs = (PartitionSpec("core"),) * (n_params + n_outs)
    out_specs = (PartitionSpec("core"),) * n_outs
    donate = tuple(range(n_params, n_params + n_outs))
    sharded = jax.jit(
        shard_map(_body, mesh=mesh, in_specs=in_specs,
                  out_specs=out_specs, check_rep=False),
        donate_argnums=donate, keep_unused=True)

    shard = NamedSharding(mesh, PartitionSpec("core"))
    zmaker = jax.jit(
        lambda: tuple(
            jnp.zeros((n_cores * a.shape[0], *a.shape[1:]), a.dtype)
            for a in out_avals),
        out_shardings=(shard,) * n_outs)

    def dispatch(in_maps):
        concat_in = [
            np.concatenate([np.asarray(m[nm]) for m in in_maps], axis=0)
            for nm in in_names]
        out_arrs = sharded(*concat_in, *zmaker())
        return [
            {nm: np.asarray(out_arrs[i]).reshape(
                n_cores, *out_avals[i].shape)[c]
             for i, nm in enumerate(out_names)}
            for c in range(n_cores)]

    _CACHE["dispatch"] = dispatch
    return dispatch


def kernel(**inputs):
    dispatch = _get_dispatch()
    in_maps = host_prep(inputs)
    res = dispatch(in_maps)
    return host_post(res, inputs.get("bo"))


if __name__ == "__main__":
    _get_compiled()
    print("build + compile OK")


# revision 9
# speedup vs baseline: 17.1393x; 2.1028x over previous
"""Trainium2 Bass kernel for 3D deformable attention (8 NeuronCores).

Sharding: core c handles (b, mq) = (c // 4, c % 4): batch b, query
quarter mq (1024 of 4096 queries).  Each core runs the full offset /
sampling branch (all 4 groups, replicated within a batch), attention for
all 8 heads over its own 1024 queries, and the full output projection
y[:, mq-slice] = wo @ out + (wo @ bv + bo).  The host only concatenates
the 8 disjoint output slices — no summation, no bias.

All per-core data is packed into ONE f32 "blob" input [128, 4522]:
Q_feature[b] and KV_feature[b] bf16-packed (cols 0:2048 / 2048:4096),
the five [128,128] weight matrices bf16-packed (cols 4096:4416), and
small f32 constants after that.  The per-core query-quarter selection is
data-driven (a one-hot sel4 column in the blob), so a single SPMD NEFF
serves all 8 cores.

On-device builds (to minimize axon-tunnel upload bytes):
 - the x-interleaved trilinear gather table kvt [18432, 64] is built
   from the raw KV feature via 32 PE transposes + 258 strided DMAs
   (was a 4.7 MB host-precomputed ExternalInput);
 - the 27 depthwise-conv diagonal matrices are expanded from the raw
   [128, 27] taps with tensor_scalar against an identity (was 1.77 MB).

Numerical notes vs the jax reference:
 - bk is dropped: a per-(head,query) constant shift of attention logits
   is softmax-invariant.
 - bv enters as wo @ bv folded into the output bias (attention weights
   sum to 1 after normalization).
 - softmax skips the max-subtraction (logits are O(0.3)).
 - gelu(exact-erf) is replaced by the tanh approximation, with tanh and
   LayerNorm's rsqrt computed from exp/ln so one ACT table set serves
   the whole kernel.
 - inputs, the five big weight matrices, and the output are bf16.
"""

import math
import sys

for _p in ("/opt/trn_rl_repo",):
    if _p not in sys.path:
        sys.path.insert(0, _p)

import numpy as np
import ml_dtypes

import concourse.bass as bass
import concourse.mybir as mybir
import concourse.tile as tile
from concourse import bacc
from concourse.masks import make_identity

F32 = mybir.dt.float32
BF16 = mybir.dt.bfloat16
I32 = mybir.dt.int32
I16 = mybir.dt.int16
AF = mybir.ActivationFunctionType
ALU = mybir.AluOpType
NPBF16 = ml_dtypes.bfloat16

B = 2
CH = 128
HEADS = 8
GROUPS = 4
GC = CH // GROUPS     # 32
HC = CH // HEADS      # 16
SP = 16
NQ = SP * SP * SP     # 4096
QPC = NQ // 4         # 1024 queries per core
DK = 8
NS = DK * DK * DK     # 512 samples per group
KS = 3
EPS = 1e-5
SCALE = HC ** -0.5
XSLOTS = SP + 2       # x slots represent x = -1 .. 16 (18 slots)
G_ROWS = SP * SP * XSLOTS  # 4608 gather rows per group
N_IDX = GROUPS * 4 * NS    # 8192 gather descriptors
GELU_C = 0.044715
GELU_S = math.sqrt(2.0 / math.pi)
NCORES = 8

# ---- blob column layout (f32 columns) -------------------------------
# cols 0:1024 are this core's query-quarter of Q and KV, bf16-packed;
# they are AllGather'd across the 4-core batch group on device.
O_QKQ = 0             # 512 cols qf quarter + 512 cols kv quarter (bf16)
O_W5 = 1024           # 5 * 64 cols: wq_t, wk_t, wv_t, wo_sA, wo_sB (bf16)
O_WDW = O_W5 + 5 * 64         # [128, 27] conv taps
O_BQ = O_WDW + 27
O_BDW = O_BQ + 1
O_LNW = O_BDW + 1
O_LNWN = O_LNW + 1
O_LNB = O_LNWN + 1
O_YBO = O_LNB + 1
O_PROJ = O_YBO + 1            # [128, 12]
O_MEAN = O_PROJ + 12          # [128, 4] mean lhsT
O_BCT = O_MEAN + 4            # [128, 4] bcast lhsT transposed
O_BC4 = O_BCT + 4             # [128, 4] denominator-bcast lhsT, transp
O_GOFF = O_BC4 + 4            # [4(rows), 1] group row offsets
O_RXYZ = O_GOFF + 1           # 48 cols, rxyz[r, q*128+p] at col r*4+q
W_BLOB = O_RXYZ + 48          # 1450
N_SM = W_BLOB - O_WDW         # 106 small f32 columns kept past scratch


# ============================================================ host prep

def _np(x):
    return np.ascontiguousarray(np.asarray(x, dtype=np.float32))


def _pack16(a):
    """[128, n] f32 -> bf16 -> view as [128, n//2] f32 container."""
    b16 = np.ascontiguousarray(a.astype(NPBF16))
    return b16.view(np.float32)


def host_prep(inp):
    """inp: dict of full numpy inputs. Returns in_maps (one blob per core)."""
    Qf = _np(inp["Q_feature"]).reshape(B, CH, NQ)
    KVf = _np(inp["KV_feature"]).reshape(B, CH, NQ)
    wq = _np(inp["wq"]); bq = _np(inp["bq"])
    w_off_dw = _np(inp["w_off_dw"]); b_off_dw = _np(inp["b_off_dw"])
    ln_w = _np(inp["ln_w"]); ln_b = _np(inp["ln_b"])
    w_off_proj = _np(inp["w_off_proj"])
    wk = _np(inp["wk"]); wv = _np(inp["wv"]); bv = _np(inp["bv"])
    wo = _np(inp["wo"]); bo = _np(inp["bo"])

    com = np.zeros((CH, W_BLOB - O_W5), np.float32)  # shared weight columns

    def put(off, arr):
        arr = np.asarray(arr, np.float32)
        com[:arr.shape[0], off - O_W5:off - O_W5 + arr.shape[1]] = arr

    put(O_W5 + 0 * 64, _pack16(wq.T))
    put(O_W5 + 1 * 64, _pack16((wk * SCALE).T))
    put(O_W5 + 2 * 64, _pack16(wv.T))
    wo_s = []
    for AB in range(2):
        m = np.zeros((CH, CH), np.float32)
        for h4 in range(4):
            h = AB * 4 + h4
            m[32 * h4 + 1: 32 * h4 + 17, :] = wo[:, HC * h: HC * (h + 1)].T
        wo_s.append(m)
    put(O_W5 + 3 * 64, _pack16(wo_s[0]))
    put(O_W5 + 4 * 64, _pack16(wo_s[1]))

    wdw = w_off_dw.reshape(GC, KS ** 3)
    put(O_WDW, np.tile(wdw, (GROUPS, 1)))          # [128, 27]
    put(O_BQ, bq.reshape(CH, 1))
    put(O_BDW, np.tile(b_off_dw, GROUPS).reshape(CH, 1))
    put(O_LNW, np.tile(ln_w, GROUPS).reshape(CH, 1))
    put(O_LNWN, -np.tile(ln_w, GROUPS).reshape(CH, 1))
    put(O_LNB, np.tile(ln_b, GROUPS).reshape(CH, 1))
    put(O_YBO, (wo @ bv + bo).reshape(CH, 1))

    projw_neg = np.zeros((CH, 12), np.float32)
    for j in range(GROUPS):
        for ax in range(3):
            projw_neg[j * GC:(j + 1) * GC, ax * 4 + j] = -w_off_proj[ax]
    put(O_PROJ, projw_neg)

    blk = np.zeros((CH, GROUPS), np.float32)       # block membership
    for j in range(GROUPS):
        blk[j * GC:(j + 1) * GC, j] = 1.0
    put(O_MEAN, blk / GC)
    put(O_BCT, blk)                                # bcast lhsT, transposed
    bc4 = np.zeros((CH, GROUPS), np.float32)
    for j in range(GROUPS):
        bc4[32 * j + 1:32 * j + 17, j] = 1.0
    put(O_BC4, bc4)                                # denom bcast, transposed

    goff = np.zeros((CH, 1), np.float32)
    for j in range(GROUPS):
        goff[j, 0] = 1.0 + j * G_ROWS
    put(O_GOFF, goff)

    # sampling reference grid (z, y, x): rxyz[ax*4+j, s] identical over j
    r = (np.linspace(0.5, DK - 0.5, DK, dtype=np.float32) / DK) * 2 - 1
    zz, yy, xx = np.meshgrid(r, r, r, indexing="ij")
    axes = [zz.reshape(NS), yy.reshape(NS), xx.reshape(NS)]
    rxyz = np.zeros((12, NS), np.float32)
    for ax in range(3):
        for j in range(GROUPS):
            rxyz[ax * 4 + j] = (axes[ax] + 1.0) * 7.5 + 1.875
    rpk = np.zeros((CH, 48), np.float32)
    for rr in range(12):
        for q in range(4):
            rpk[:, rr * 4 + q] = rxyz[rr, q * 128:(q + 1) * 128]
    put(O_RXYZ, rpk)

    in_maps = []
    for c in range(NCORES):
        b, mq = c // 4, c % 4
        blob = np.zeros((CH, W_BLOB), np.float32)
        blob[:, 0:512] = _pack16(Qf[b][:, mq * QPC:(mq + 1) * QPC])
        blob[:, 512:1024] = _pack16(KVf[b][:, mq * QPC:(mq + 1) * QPC])
        blob[:, O_W5:] = com
        in_maps.append({"blob": blob})
    return in_maps


def host_post(results, bo=None):
    """results: list of 8 dicts with 'py16' [128, 1024] bf16."""
    y = np.zeros((B, CH, NQ), np.float32)
    for c in range(NCORES):
        b, mq = c // 4, c % 4
        y[b][:, mq * QPC:(mq + 1) * QPC] = np.asarray(
            results[c]["py16"]).astype(np.float32)
    return y.reshape(B, CH, SP, SP, SP)


# ============================================================ device build

def build_program(tc: tile.TileContext, ctx):
    nc = tc.nc

    blob = nc.dram_tensor("blob", [CH, W_BLOB], F32, kind="ExternalInput").ap()
    py16 = nc.dram_tensor("py16", [CH, QPC], BF16, kind="ExternalOutput").ap()

    kvt_dram = nc.dram_tensor("kvt_dram", [GROUPS * G_ROWS, 2 * GC], F32).ap()
    qkq_dram = nc.dram_tensor("qkq_dram", [CH, 1024], F32).ap()
    qkall_dram = nc.dram_tensor("qkall_dram", [4 * CH, 1024], F32).ap()
    idx_dram = nc.dram_tensor("idx_dram", [N_IDX], I16).ap()
    co_dram = nc.dram_tensor("co_dram", [12 * 3 * NS], F32).ap()
    w8_dram = nc.dram_tensor("w8_dram", [64 * 2 * CH], F32).ap()

    consts = ctx.enter_context(tc.tile_pool(name="consts", bufs=1))
    live = ctx.enter_context(tc.tile_pool(name="live", bufs=1))

    ident = consts.tile([CH, CH], F32, tag="ident", name="ident")
    make_identity(nc, ident[:])

    # small f32 constants that must outlive the scratch phase
    smalls = consts.tile([CH, N_SM], F32, tag="smalls", name="smalls")

    def bcol(off, n=1):
        return smalls[:, off - O_WDW:off - O_WDW + n]

    # [row-dim < 128] constants come via rearranged DMA straight from DRAM
    bcast_sb = consts.tile([GROUPS, CH], F32, tag="bcast_sb", name="bcast_sb")
    nc.sync.dma_start(bcast_sb[:],
                      blob[:, O_BCT:O_BCT + 4].rearrange("p j -> j p"))
    rxyz_sb = consts.tile([12, NS], F32, tag="rxyz_sb", name="rxyz_sb")
    nc.sync.dma_start(
        rxyz_sb[:].rearrange("r (q p) -> r q p", q=4),
        blob[:, O_RXYZ:O_RXYZ + 48].rearrange("p (r q) -> r q p", r=12))
    goff_sb = consts.tile([GROUPS, 1], F32, tag="goff_sb", name="goff_sb")
    nc.sync.dma_start(goff_sb[:], blob[0:GROUPS, O_GOFF:O_GOFF + 1])
    bq8 = consts.tile([HC, HEADS], F32, tag="bq8", name="bq8")
    nc.sync.dma_start(
        bq8[:], blob[:, O_BQ:O_BQ + 1].rearrange("(h c) j -> c (h j)", h=8))

    # bc4 lhsT for denominator broadcast: rows j, cols 32j+1..32j+17 = 1
    bc4_sb = consts.tile([GROUPS, CH], F32, tag="bc4_sb", name="bc4_sb")
    nc.sync.dma_start(bc4_sb[:],
                      blob[:, O_BC4:O_BC4 + 4].rearrange("p j -> j p"))

    # five bf16-packed [128,128] matrices -> f32 SBUF tiles
    w5 = consts.tile([CH, 5 * CH], F32, tag="w5", name="w5")
    wq_sb = w5[:, 0 * CH:1 * CH]
    wk_sb = w5[:, 1 * CH:2 * CH]
    wv_sb = w5[:, 2 * CH:3 * CH]
    woA_sb = w5[:, 3 * CH:4 * CH]
    woB_sb = w5[:, 4 * CH:5 * CH]

    convw_sb = consts.tile([CH, KS ** 3 * CH], F32, tag="convw_sb",
                           name="convw_sb")

    # tiles that outlive the scratch phases
    qsl = live.tile([CH, QPC], F32, tag="qsl", name="qsl")
    vt_sb = live.tile([CH, 4 * 2 * 4 * GC], F32, tag="vt_sb", name="vt_sb")
    xs_sb = live.tile([CH, GROUPS * CH], F32, tag="xs_sb", name="xs_sb")

    with tc.tile_pool(name="scratch", bufs=1) as scr, \
         tc.tile_pool(name="ps", bufs=2, space="PSUM") as ps_pool, \
         tc.tile_pool(name="pst", bufs=1, space="PSUM") as pst_pool:

        _cnt = [0]

        def st(tag, shape=(CH, NS), dt=F32):
            _cnt[0] += 1
            return scr.tile(list(shape), dt, tag=tag, name=f"{tag}_{_cnt[0]}")

        def ps512():
            return ps_pool.tile([CH, 512], F32, tag="ps512", name="ps512")

        # ---- phase 0: load blob, unpack, build conv diagonals --------
        blob_sb = st("blob", (CH, W_BLOB))
        nc.sync.dma_start(blob_sb[:], blob)
        nc.vector.tensor_copy(smalls[:], blob_sb[:, O_WDW:W_BLOB])
        nc.scalar.activation(
            w5[:], blob_sb[:, O_W5:O_W5 + 5 * 64].bitcast(BF16), AF.Identity)
        for t in range(KS ** 3):
            nc.vector.tensor_scalar(convw_sb[:, t * CH:(t + 1) * CH],
                                    ident[:], bcol(O_WDW + t), None, ALU.mult)

        # AllGather the four query-quarters of Q/KV within the batch group
        nc.sync.dma_start(qkq_dram, blob_sb[:, 0:1024])
        nc.gpsimd.collective_compute(
            "AllGather", ALU.bypass,
            replica_groups=[[0, 1, 2, 3], [4, 5, 6, 7]],
            ins=[qkq_dram], outs=[qkall_dram])
        qk_sb = st("qk", (CH, NQ))
        nc.sync.dma_start(
            qk_sb[:].rearrange("p (r c) -> p r c", r=4),
            qkall_dram.rearrange("(r p) c -> p r c", r=4))
        qf_sb = st("qf", (CH, NQ))
        kv_sb = st("kv", (CH, NQ))
        for r in range(4):
            nc.scalar.activation(
                qf_sb[:, r * QPC:(r + 1) * QPC],
                qk_sb[:, r * 1024:r * 1024 + 512].bitcast(BF16), AF.Identity)
            nc.scalar.activation(
                kv_sb[:, r * QPC:(r + 1) * QPC],
                qk_sb[:, r * 1024 + 512:(r + 1) * 1024].bitcast(BF16),
                AF.Identity)
        # this core's own 1024 queries (for attention Q)
        nc.scalar.activation(qsl[:], blob_sb[:, 0:512].bitcast(BF16),
                             AF.Identity)

        # transpose kv to spatial-major: tile k holds [sp 128k.., ch]
        kvT_sb = st("kvT", (CH, NQ))
        for k in range(NQ // CH):
            pt = ps512()
            nc.tensor.transpose(pt[:, 0:CH], kv_sb[:, k * CH:(k + 1) * CH],
                                ident[:])
            nc.scalar.activation(kvT_sb[:, k * CH:(k + 1) * CH], pt[:, 0:CH],
                                 AF.Identity)

        # scatter into the x-interleaved gather table
        # row (g, z, y, xs) = [KV[g,:,z,y,xs-1], KV[g,:,z,y,xs]]
        kvt_v = kvt_dram.rearrange("(g z y xs) w -> g z y xs w",
                                   g=GROUPS, z=SP, y=SP)
        for k in range(NQ // CH):
            z, yh = k // 2, k % 2
            for g in range(GROUPS):
                src = kvT_sb[:, k * CH + g * GC:k * CH + (g + 1) * GC]
                nc.sync.dma_start(
                    kvt_v[g, z, yh * 8:yh * 8 + 8, 1:SP + 1, 0:GC], src)
                nc.sync.dma_start(
                    kvt_v[g, z, yh * 8:yh * 8 + 8, 0:SP, GC:2 * GC], src)
        # finite-fill the two fetched-but-masked edge columns
        zfill = st("zfill", (CH, 256))
        nc.vector.memset(zfill[:], 0.0)
        nc.sync.dma_start(
            kvt_v[:, :, :, 0, 0:GC].rearrange("g z y c -> (g z y) c")
            .rearrange("(a b) c -> a b c", a=CH),
            zfill[:].rearrange("p (b c) -> p b c", c=GC))
        nc.sync.dma_start(
            kvt_v[:, :, :, SP, GC:2 * GC].rearrange("g z y c -> (g z y) c")
            .rearrange("(a b) c -> a b c", a=CH),
            zfill[:].rearrange("p (b c) -> p b c", c=GC))

        # ---- phase 1: Q projection -----------------------------------
        # full projection into a zero-padded 17^3 buffer (conv input)
        SPP = SP + 1
        q_pad = st("qpad", (CH, SPP ** 3))
        nc.gpsimd.memset(q_pad[:], 0.0)
        qp_zyx = q_pad[:].rearrange("p (z y x) -> p z y x", z=SPP, y=SPP)
        for i in range(NQ // 512):   # chunk = 2 z-slabs
            pq = ps512()
            nc.tensor.matmul(pq[:], wq_sb,
                             qf_sb[:, i * 512:(i + 1) * 512],
                             start=True, stop=True)
            nc.scalar.activation(
                qp_zyx[:, 1 + 2 * i:3 + 2 * i, 1:SP + 1, 1:SP + 1],
                pq[:].rearrange("p (a b c) -> p a b c", a=2, b=SP),
                AF.Identity, bias=bcol(O_BQ))

        # ---- phase 2: depthwise conv (stride 2) ----------------------
        pc = pst_pool.tile([CH, NS], F32, tag="psA", name="psA")
        first = True
        for dz in range(KS):
            for dy in range(KS):
                for dx in range(KS):
                    t = (dz * KS + dy) * KS + dx
                    rhs = qp_zyx[:, dz:dz + 2 * DK - 1:2,
                                 dy:dy + 2 * DK - 1:2,
                                 dx:dx + 2 * DK - 1:2]
                    nc.tensor.matmul(pc[:], convw_sb[:, t * CH:(t + 1) * CH],
                                     rhs, start=first,
                                     stop=(t == KS ** 3 - 1))
                    first = False
        c_sb = st("s0")
        nc.scalar.activation(c_sb[:], pc[:], AF.Identity, bias=bcol(O_BDW))

        # ---- phase 3: LayerNorm over 32-channel blocks ---------------
        csq = st("s1")
        nc.scalar.activation(csq[:], c_sb[:], AF.Square)
        pmu = pst_pool.tile([GROUPS, NS], F32, tag="psB", name="psB")
        nc.tensor.matmul(pmu[:], bcol(O_MEAN, 4), c_sb[:],
                         start=True, stop=True)
        pmsq = pst_pool.tile([GROUPS, NS], F32, tag="psC", name="psC")
        nc.tensor.matmul(pmsq[:], bcol(O_MEAN, 4), csq[:],
                         start=True, stop=True)
        mu2 = st("s2", (GROUPS, NS))
        nc.scalar.activation(mu2[:], pmu[:], AF.Square)
        var = st("s3", (GROUPS, NS))
        nc.vector.tensor_sub(var[:], pmsq[:], mu2[:])
        eps_sb = st("eps", (GROUPS, 1))
        nc.vector.memset(eps_sb[:], EPS)
        lnv = st("s2b", (GROUPS, NS))
        nc.scalar.activation(lnv[:], var[:], AF.Ln, bias=eps_sb[:])
        rstd = st("s1b", (GROUPS, NS))
        nc.scalar.activation(rstd[:], lnv[:], AF.Exp, scale=-0.5)
        murstd = st("s3b", (GROUPS, NS))
        nc.vector.tensor_mul(murstd[:], pmu[:], rstd[:])
        prb = pst_pool.tile([CH, NS], F32, tag="psA", name="psA2")
        nc.tensor.matmul(prb[:], bcast_sb[:], rstd[:], start=True, stop=True)
        pmb = pst_pool.tile([CH, NS], F32, tag="psA", name="psA3")
        nc.tensor.matmul(pmb[:], bcast_sb[:], murstd[:], start=True, stop=True)
        a_bc = st("s2")
        nc.vector.tensor_scalar(a_bc[:], prb[:], bcol(O_LNW), None, ALU.mult)
        b_bc = st("s3")
        nc.vector.tensor_scalar(b_bc[:], pmb[:], bcol(O_LNWN), bcol(O_LNB),
                                ALU.mult, ALU.add)
        u = st("s4")
        nc.vector.tensor_mul(u[:], c_sb[:], a_bc[:])
        nc.vector.tensor_add(u[:], u[:], b_bc[:])

        # ---- phase 4: gelu (tanh approx; tanh via exp) ---------------
        usq = st("s0")
        nc.scalar.activation(usq[:], u[:], AF.Square)
        ucb = st("s1")
        nc.vector.tensor_mul(ucb[:], usq[:], u[:])
        g2 = st("s2")
        nc.vector.scalar_tensor_tensor(g2[:], ucb[:], GELU_C, u[:],
                                       ALU.mult, ALU.add)
        ge = st("s3")
        nc.scalar.activation(ge[:], g2[:], AF.Exp, scale=2.0 * GELU_S)
        nc.vector.tensor_scalar(ge[:], ge[:], 1.0, None, ALU.add)
        gr = st("s0")
        nc.vector.reciprocal(gr[:], ge[:])
        gneg = st("s1")
        nc.vector.scalar_tensor_tensor(gneg[:], gr[:], 1.0, u[:],
                                       ALU.subtract, ALU.mult)  # -gelu

        # ---- phase 5: offset proj + coords ---------------------------
        poff = pst_pool.tile([12, NS], F32, tag="psB", name="psB2")
        nc.tensor.matmul(poff[:], bcol(O_PROJ, 12), gneg[:],
                         start=True, stop=True)
        ce = st("s2", (12, NS))
        nc.scalar.activation(ce[:], poff[:], AF.Exp, scale=2.0)
        nc.vector.tensor_scalar(ce[:], ce[:], 1.0, None, ALU.add)
        cr = st("s3", (12, NS))
        nc.vector.reciprocal(cr[:], ce[:])
        ixyz = st("s4", (12, NS))
        nc.vector.scalar_tensor_tensor(ixyz[:], cr[:], -3.75, rxyz_sb[:],
                                       ALU.mult, ALU.add)
        ci = st("s0", (12, NS), I32)
        nc.vector.tensor_copy(ci[:], ixyz[:])
        cf = st("s1", (12, NS))
        nc.vector.tensor_copy(cf[:], ci[:])
        fixm = st("s2", (12, NS))
        nc.vector.tensor_tensor(fixm[:], cf[:], ixyz[:], ALU.is_gt)
        f0 = st("s5", (12, NS))
        nc.vector.tensor_sub(f0[:], cf[:], fixm[:])
        tfrac = st("s3", (12, NS))
        nc.vector.tensor_sub(tfrac[:], ixyz[:], f0[:])
        m0 = st("s0", (12, NS))
        nc.vector.tensor_scalar(m0[:], f0[:], 0.0, None, ALU.is_ge)
        m1 = st("s1", (12, NS))
        nc.vector.tensor_scalar(m1[:], f0[:], 14.0, None, ALU.is_le)
        omt = st("s2", (12, NS))
        nc.vector.tensor_scalar(omt[:], tfrac[:], -1.0, 1.0, ALU.mult, ALU.add)

        big = st("big", (12, 3 * NS))
        nc.vector.tensor_copy(big[:, 0:NS], f0[:])
        nc.vector.tensor_mul(big[:, NS:2 * NS], omt[:], m0[:])
        nc.vector.tensor_mul(big[:, 2 * NS:3 * NS], tfrac[:], m1[:])
        nc.sync.dma_start(co_dram, big[:])
        co_g = st("co_g", (GROUPS, 9 * NS))
        nc.sync.dma_start(
            co_g[:].rearrange("g (ax k s) -> g ax k s", ax=3, k=3),
            co_dram.rearrange("(ax g k s) -> g ax k s", ax=3, g=4, k=3))

        def cgs(ax, kind):  # kind: 0 = floor, 1 = w0, 2 = w1
            o = (ax * 3 + kind) * NS
            return co_g[:, o:o + NS]

        zc0 = st("s0", (GROUPS, NS))
        zc1 = st("s1", (GROUPS, NS))
        yc0 = st("s2", (GROUPS, NS))
        yc1 = st("s3", (GROUPS, NS))
        nc.vector.tensor_scalar(zc0[:], cgs(0, 0), 0.0, 15.0, ALU.max, ALU.min)
        nc.vector.tensor_scalar(zc1[:], cgs(0, 0), 1.0, 0.0, ALU.add, ALU.max)
        nc.vector.tensor_scalar(zc1[:], zc1[:], 15.0, None, ALU.min)
        nc.vector.tensor_scalar(yc0[:], cgs(1, 0), 0.0, 15.0, ALU.max, ALU.min)
        nc.vector.tensor_scalar(yc1[:], cgs(1, 0), 1.0, 0.0, ALU.add, ALU.max)
        nc.vector.tensor_scalar(yc1[:], yc1[:], 15.0, None, ALU.min)
        xoff2 = st("s4", (GROUPS, NS))
        nc.vector.tensor_scalar(xoff2[:], cgs(2, 0), goff_sb[:], None, ALU.add)

        idxf = st("s5", (GROUPS, NS))
        idx16 = st("idx16", (GROUPS, 4 * NS), I16)
        wzy = st("wzy", (GROUPS, 4 * NS))
        zcs, ycs = [zc0, zc1], [yc0, yc1]
        for a in range(2):
            for bb in range(2):
                zy = a * 2 + bb
                nc.vector.scalar_tensor_tensor(
                    idxf[:], zcs[a][:], float(SP * XSLOTS), xoff2[:],
                    ALU.mult, ALU.add)
                nc.vector.scalar_tensor_tensor(
                    idxf[:], ycs[bb][:], float(XSLOTS), idxf[:],
                    ALU.mult, ALU.add)
                nc.vector.tensor_scalar(idxf[:], idxf[:], 0.0,
                                        float(GROUPS * G_ROWS - 1),
                                        ALU.max, ALU.min)
                nc.vector.tensor_copy(idx16[:, zy * NS:(zy + 1) * NS], idxf[:])
                nc.vector.tensor_mul(wzy[:, zy * NS:(zy + 1) * NS],
                                     cgs(0, 1 + a), cgs(1, 1 + bb))
        nc.sync.dma_start(idx_dram, idx16[:])
        # full trilinear corner weights w8[g, zy, x, s] = wzy * wx,
        # streamed piecewise to DRAM addressed (g zy si p x)
        w8d = w8_dram.rearrange("(g zy si p x) -> g zy si p x",
                                g=4, zy=4, si=4, x=2)
        for zy in range(4):
            for x in range(2):
                w8p = st("w8p", (GROUPS, NS))
                nc.vector.tensor_mul(w8p[:],
                                     wzy[:, zy * NS:(zy + 1) * NS],
                                     cgs(2, 1 + x))
                nc.sync.dma_start(
                    w8d[:, zy, :, :, x],
                    w8p[:].rearrange("g (si p) -> g si p", si=4))

        # wrapped idx [128, 512]: global idx i at (i%16, i//16), x8 blocks
        idxw = st("idxw", (CH, N_IDX // 16), I16)
        for rep in range(8):
            nc.gpsimd.dma_start(
                idxw[rep * 16:(rep + 1) * 16, :],
                idx_dram.rearrange("(col r) -> r col", r=16))

        # ---- phase 6: gather + trilinear combine ---------------------
        gth = scr.tile([CH, N_IDX // CH, 2 * GC], F32, tag="gth", name="gth")
        NCHK = 32
        CH_I = N_IDX // NCHK          # 256 idx per gather call
        for k in range(NCHK):
            nc.gpsimd.dma_gather(
                out_ap=gth[:, k * (CH_I // 128):(k + 1) * (CH_I // 128), :],
                in_ap=kvt_dram,
                idxs_ap=idxw[:, k * (CH_I // 16):(k + 1) * (CH_I // 16)],
                num_idxs=CH_I, num_idxs_reg=CH_I, elem_size=2 * GC)

        # stream order: i = ((g*4 + zy)*4 + si)*128 + p, sample s = si*128+p
        w8b = scr.tile([CH, 64, 2], F32, tag="tB", name="w8b")
        nc.sync.dma_start(
            w8b[:],
            w8_dram.rearrange("(j p x) -> p j x", j=64, x=2))
        nc.vector.tensor_tensor(
            gth[:].rearrange("p j (x c) -> p j x c", x=2),
            gth[:].rearrange("p j (x c) -> p j x c", x=2),
            w8b[:].unsqueeze(3).broadcast_to([CH, 64, 2, GC]), ALU.mult)
        t2v = gth[:].rearrange("p (g zy si) e -> p g zy (si e)", g=4, zy=4)
        sa = st("sa", (CH, GROUPS, 4 * 2 * GC))
        sb = st("sb", (CH, GROUPS, 4 * 2 * GC))
        nc.vector.tensor_tensor(sa[:], t2v[:, :, 0], t2v[:, :, 1], ALU.add)
        nc.vector.tensor_tensor(sb[:], t2v[:, :, 2], t2v[:, :, 3], ALU.add)
        nc.vector.tensor_tensor(sa[:], sa[:], sb[:], ALU.add)
        sav = sa[:].rearrange("p g (si x c) -> p g si x c", si=4, x=2)
        xs_t = st("s0", (CH, 4, GROUPS, GC))   # [p, si, g, c]
        nc.vector.tensor_tensor(xs_t[:].rearrange("p si g c -> p g si c"),
                                sav[:, :, :, 0, :],
                                sav[:, :, :, 1, :], ALU.add)

        # ---- phase 7: transpose to xs [128 (g,c), 512 n] -------------
        for si in range(4):
            pt = ps512()
            nc.tensor.transpose(
                pt[:, 0:CH], xs_t[:, si].rearrange("p g c -> p (g c)"),
                ident[:])
            nc.scalar.activation(xs_sb[:, si * CH:(si + 1) * CH], pt[:, 0:CH],
                                 AF.Identity)

        # ---- phase 8: V-hat ------------------------------------------
        nc.vector.memset(vt_sb[:], 0.0)
        nc.vector.memset(
            vt_sb[:].rearrange("p (n h s) -> p n h s", n=4, h=HEADS)
            [:, :, :, 0:1], 1.0)
        for nch in range(4):
            pv = ps512()
            nc.tensor.matmul(pv[:, 0:CH],
                             xs_sb[:, nch * CH:(nch + 1) * CH],
                             wv_sb, start=True, stop=True)
            nc.vector.tensor_copy(
                vt_sb[:].rearrange("p (n h s) -> p n h s", n=4, h=HEADS)
                [:, nch, :, 1:HC + 1],
                pv[:, 0:CH].rearrange("p (h c) -> p h c", h=HEADS))
        # (vt layout per n-chunk: 8 x [1 | V(16) | 0*15], 256 wide)

    # ---- phase 9: attention loop -------------------------------------
    # per query-half: Q/K per head on the fly, logits -> exp -> V-hat
    # accumulation (denominator in row 0 of each 32-block), normalize,
    # project through wo and emit bf16.
    with tc.tile_pool(name="pA", bufs=2, space="PSUM") as pA, \
         tc.tile_pool(name="pO", bufs=2, space="PSUM") as pO, \
         tc.tile_pool(name="pR", bufs=1, space="PSUM") as pR, \
         tc.tile_pool(name="pY", bufs=1, space="PSUM") as pY, \
         tc.tile_pool(name="att_pool", bufs=3) as att_pool, \
         tc.tile_pool(name="opool", bufs=2) as opool:
        for qh in range(2):
            on_tiles = {}
            for AB in range(2):
                po = pO.tile([CH, 512], F32, tag="po", name="po")
                for h4 in range(4):
                    h = AB * 4 + h4
                    pq2 = pA.tile([HC, 512], F32, tag="p16", name="p16q")
                    nc.tensor.matmul(pq2[:], wq_sb[:, HC * h:HC * (h + 1)],
                                     qsl[:, qh * 512:(qh + 1) * 512],
                                     start=True, stop=True)
                    q2h = att_pool.tile([HC, 512], F32, tag="q2h", name="q2h")
                    nc.vector.tensor_scalar(q2h[:], pq2[:], bq8[:, h:h + 1],
                                            None, ALU.add)
                    pk = pA.tile([HC, 512], F32, tag="p16", name="p16k")
                    nc.tensor.matmul(pk[:], wk_sb[:, HC * h:HC * (h + 1)],
                                     xs_sb[:], start=True, stop=True)
                    k2h = att_pool.tile([HC, 512], F32, tag="k2h", name="k2h")
                    nc.scalar.activation(k2h[:], pk[:], AF.Identity)
                    for nch in range(4):
                        pa = pA.tile([CH, 512], F32, tag="pa", name="pa")
                        nc.tensor.matmul(
                            pa[:], k2h[:, nch * CH:(nch + 1) * CH], q2h[:],
                            start=True, stop=True)
                        att = att_pool.tile([CH, 512], F32, tag="att",
                                            name="att")
                        nc.scalar.activation(att[:], pa[:], AF.Exp)
                        nc.tensor.matmul(
                            po[32 * h4:32 * h4 + 32, :],
                            vt_sb[:, nch * 256 + h * 32:
                                  nch * 256 + h * 32 + 32],
                            att[:], start=(nch == 0), stop=(nch == 3),
                            skip_group_check=True,
                            tile_position=(0, 32 * h4))
                o_sb = opool.tile([CH, 512], F32, tag="o_sb", name="o_sb")
                nc.scalar.activation(o_sb[:], po[:], AF.Identity)
                den4 = opool.tile([GROUPS, 512], F32, tag="den4", name="den4")
                for j in range(4):
                    nc.sync.dma_start(den4[j:j + 1, :],
                                      o_sb[32 * j:32 * j + 1, :])
                rd4 = opool.tile([GROUPS, 512], F32, tag="rd4", name="rd4")
                nc.vector.reciprocal(rd4[:], den4[:])
                prd = pR.tile([CH, 512], F32, tag="prd", name="prd")
                nc.tensor.matmul(prd[:], bc4_sb[:], rd4[:],
                                 start=True, stop=True)
                on_sb = opool.tile([CH, 512], F32, tag=f"on{AB}",
                                   name=f"on{AB}")
                nc.vector.tensor_mul(on_sb[:], o_sb[:], prd[:])
                on_tiles[AB] = on_sb

            pyp = pY.tile([CH, 512], F32, tag="pyp", name="pyp")
            for AB in range(2):
                nc.tensor.matmul(pyp[:],
                                 (woA_sb if AB == 0 else woB_sb),
                                 on_tiles[AB][:],
                                 start=(AB == 0), stop=(AB == 1))
            y16 = opool.tile([CH, 512], BF16, tag="y16", name="y16")
            nc.scalar.activation(y16[:], pyp[:], AF.Identity,
                                 bias=bcol(O_YBO))
            nc.sync.dma_start(py16[:, qh * 512:(qh + 1) * 512], y16[:])


# ============================================================ entry points

_CACHE = {}


def _get_compiled():
    if "nc" in _CACHE:
        return _CACHE["nc"]
    from contextlib import ExitStack
    nc = bacc.Bacc("TRN2", target_bir_lowering=False, debug=False,
                   num_devices=NCORES)
    with tile.TileContext(nc) as tc:
        with ExitStack() as ctx:
            build_program(tc, ctx)
    nc.compile()
    _CACHE["nc"] = nc
    return nc


def _get_dispatch():
    """A cached PJRT dispatcher: same semantics as
    bass2jax.run_bass_via_pjrt (fresh host inputs in, numpy outputs back),
    but the jitted shard_map callable is built ONCE and the donated output
    buffers are created on-device instead of being shipped through the
    axon tunnel every call."""
    if "dispatch" in _CACHE:
        return _CACHE["dispatch"]
    import jax
    import jax.numpy as jnp
    from jax.sharding import Mesh, PartitionSpec, NamedSharding
    from jax.experimental.shard_map import shard_map
    from concourse.bass2jax import (_bass_exec_p, install_neuronx_cc_hook,
                                    partition_id_tensor)

    nc = _get_compiled()
    install_neuronx_cc_hook()
    n_cores = NCORES
    partition_name = (nc.partition_id_tensor.name
                      if nc.partition_id_tensor else None)
    in_names, out_names, out_avals = [], [], []
    for alloc in nc.m.functions[0].allocations:
        if not isinstance(alloc, mybir.MemoryLocationSet):
            continue
        name = alloc.memorylocations[0].name
        if alloc.kind == "ExternalInput":
            if name != partition_name:
                in_names.append(name)
        elif alloc.kind == "ExternalOutput":
            out_names.append(name)
            out_avals.append(jax.core.ShapedArray(
                tuple(alloc.tensor_shape), mybir.dt.np(alloc.dtype)))
    n_params = len(in_names)
    n_outs = len(out_avals)
    all_in_names = in_names + out_names
    if partition_name is not None:
        all_in_names.append(partition_name)

    def _body(*args):
        operands = list(args)
        if partition_name is not None:
            operands.append(partition_id_tensor())
        outs = _bass_exec_p.bind(
            *operands, out_avals=tuple(out_avals),
            in_names=tuple(all_in_names), out_names=tuple(out_names),
            lowering_input_output_aliases=(),
            sim_require_finite=True, sim_require_nnan=True, nc=nc)
        return tuple(outs)

    devices = jax.devices()[:n_cores]
    mesh = Mesh(np.asarray(devices), ("core",))
    in_specs = (PartitionSpec("core"),) * (n_params + n_outs)
    out_specs = (PartitionSpec("core"),) * n_outs
    donate = tuple(range(n_params, n_params + n_outs))
    sharded = jax.jit(
        shard_map(_body, mesh=mesh, in_specs=in_specs,
                  out_specs=out_specs, check_rep=False),
        donate_argnums=donate, keep_unused=True)

    shard = NamedSharding(mesh, PartitionSpec("core"))
    zmaker = jax.jit(
        lambda: tuple(
            jnp.zeros((n_cores * a.shape[0], *a.shape[1:]), a.dtype)
            for a in out_avals),
        out_shardings=(shard,) * n_outs)

    def dispatch(in_maps):
        concat_in = [
            np.concatenate([np.asarray(m[nm]) for m in in_maps], axis=0)
            for nm in in_names]
        out_arrs = sharded(*concat_in, *zmaker())
        return [
            {nm: np.asarray(out_arrs[i]).reshape(
                n_cores, *out_avals[i].shape)[c]
             for i, nm in enumerate(out_names)}
            for c in range(n_cores)]

    _CACHE["dispatch"] = dispatch
    return dispatch


def kernel(**inputs):
    dispatch = _get_dispatch()
    in_maps = host_prep(inputs)
    res = dispatch(in_maps)
    return host_post(res, inputs.get("bo"))


if __name__ == "__main__":
    _get_compiled()
    print("build + compile OK")


# revision 11
# speedup vs baseline: 19.6544x; 1.1467x over previous
"""Trainium2 Bass kernel for 3D deformable attention (8 NeuronCores).

Sharding: core c handles (b, mq) = (c // 4, c % 4): batch b, query
quarter mq (1024 of 4096 queries).  Each core runs the full offset /
sampling branch (all 4 groups, replicated within a batch), attention for
all 8 heads over its own 1024 queries, and the full output projection
y[:, mq-slice] = wo @ out + (wo @ bv + bo).  The host only concatenates
the 8 disjoint output slices — no summation, no bias.

All per-core data is packed into ONE f32 "blob" input [128, 4522]:
Q_feature[b] and KV_feature[b] bf16-packed (cols 0:2048 / 2048:4096),
the five [128,128] weight matrices bf16-packed (cols 4096:4416), and
small f32 constants after that.  The per-core query-quarter selection is
data-driven (a one-hot sel4 column in the blob), so a single SPMD NEFF
serves all 8 cores.

On-device builds (to minimize axon-tunnel upload bytes):
 - the x-interleaved trilinear gather table kvt [18432, 64] is built
   from the raw KV feature via 32 PE transposes + 258 strided DMAs
   (was a 4.7 MB host-precomputed ExternalInput);
 - the 27 depthwise-conv diagonal matrices are expanded from the raw
   [128, 27] taps with tensor_scalar against an identity (was 1.77 MB).

Numerical notes vs the jax reference:
 - bk is dropped: a per-(head,query) constant shift of attention logits
   is softmax-invariant.
 - bv enters as wo @ bv folded into the output bias (attention weights
   sum to 1 after normalization).
 - softmax skips the max-subtraction (logits are O(0.3)).
 - gelu(exact-erf) is replaced by the tanh approximation, with tanh and
   LayerNorm's rsqrt computed from exp/ln so one ACT table set serves
   the whole kernel.
 - inputs, the five big weight matrices, and the output are bf16.
"""

import math
import sys

for _p in ("/opt/trn_rl_repo",):
    if _p not in sys.path:
        sys.path.insert(0, _p)

import numpy as np
import ml_dtypes

import concourse.bass as bass
import concourse.mybir as mybir
import concourse.tile as tile
from concourse import bacc
from concourse.masks import make_identity

F32 = mybir.dt.float32
BF16 = mybir.dt.bfloat16
I32 = mybir.dt.int32
I16 = mybir.dt.int16
AF = mybir.ActivationFunctionType
ALU = mybir.AluOpType
NPBF16 = ml_dtypes.bfloat16

B = 2
CH = 128
HEADS = 8
GROUPS = 4
GC = CH // GROUPS     # 32
HC = CH // HEADS      # 16
SP = 16
NQ = SP * SP * SP     # 4096
QPC = NQ // 4         # 1024 queries per core
DK = 8
NS = DK * DK * DK     # 512 samples per group
KS = 3
EPS = 1e-5
SCALE = HC ** -0.5
XSLOTS = SP + 2       # x slots represent x = -1 .. 16 (18 slots)
G_ROWS = SP * SP * XSLOTS  # 4608 gather rows per group
N_IDX = GROUPS * 4 * NS    # 8192 gather descriptors
GELU_C = 0.044715
GELU_S = math.sqrt(2.0 / math.pi)
NCORES = 8

# ---- blob / com column layout (f32 columns) -------------------------
# Per-core blob = [qf quarter (512, bf16-packed) | kv quarter (512) |
# this core's 107-col chunk of the shared weight block "com"].  The
# whole 1131-col region is AllGather'd across the 4-core batch group;
# com offsets below are relative to the reassembled [128, 428] block.
O_W5 = 0              # 5 * 64 cols: wq_t, wk_t, wv_t, wo_sA, wo_sB (bf16)
O_WDW = O_W5 + 5 * 64         # [128, 27] conv taps
O_BQ = O_WDW + 27
O_BDW = O_BQ + 1
O_LNW = O_BDW + 1
O_LNWN = O_LNW + 1
O_LNB = O_LNWN + 1
O_YBO = O_LNB + 1
O_PROJ = O_YBO + 1            # [128, 12]
O_MEAN = O_PROJ + 12          # [128, 4] mean lhsT
O_BCT = O_MEAN + 4            # [128, 4] bcast lhsT transposed
O_BC4 = O_BCT + 4             # [128, 4] denominator-bcast lhsT, transp
O_GOFF = O_BC4 + 4            # [4(rows), 1] group row offsets
O_RXYZ = O_GOFF + 1           # 48 cols, rxyz[r, q*128+p] at col r*4+q
COM_W = 4 * 107               # 428 (RXYZ ends at 426, 2 pad)
CHUNK = COM_W // 4            # 107 com cols uploaded per core
W_BLOB = 1024 + CHUNK + 1     # 1132 (pad col keeps it even)
GCOL = 1024 + CHUNK           # 1131 gathered cols per core


# ============================================================ host prep

def _np(x):
    return np.ascontiguousarray(np.asarray(x, dtype=np.float32))


def _pack16(a):
    """[128, n] f32 -> bf16 -> view as [128, n//2] f32 container."""
    b16 = np.ascontiguousarray(a.astype(NPBF16))
    return b16.view(np.float32)


def host_prep(inp):
    """inp: dict of full numpy inputs. Returns in_maps (one blob per core)."""
    Qf = _np(inp["Q_feature"]).reshape(B, CH, NQ)
    KVf = _np(inp["KV_feature"]).reshape(B, CH, NQ)
    wq = _np(inp["wq"]); bq = _np(inp["bq"])
    w_off_dw = _np(inp["w_off_dw"]); b_off_dw = _np(inp["b_off_dw"])
    ln_w = _np(inp["ln_w"]); ln_b = _np(inp["ln_b"])
    w_off_proj = _np(inp["w_off_proj"])
    wk = _np(inp["wk"]); wv = _np(inp["wv"]); bv = _np(inp["bv"])
    wo = _np(inp["wo"]); bo = _np(inp["bo"])

    com = np.zeros((CH, COM_W), np.float32)       # shared weight columns

    def put(off, arr):
        arr = np.asarray(arr, np.float32)
        com[:arr.shape[0], off - O_W5:off - O_W5 + arr.shape[1]] = arr

    put(O_W5 + 0 * 64, _pack16(wq.T))
    put(O_W5 + 1 * 64, _pack16((wk * SCALE).T))
    put(O_W5 + 2 * 64, _pack16(wv.T))
    wo_s = []
    for AB in range(2):
        m = np.zeros((CH, CH), np.float32)
        for h4 in range(4):
            h = AB * 4 + h4
            m[32 * h4 + 1: 32 * h4 + 17, :] = wo[:, HC * h: HC * (h + 1)].T
        wo_s.append(m)
    put(O_W5 + 3 * 64, _pack16(wo_s[0]))
    put(O_W5 + 4 * 64, _pack16(wo_s[1]))

    wdw = w_off_dw.reshape(GC, KS ** 3)
    put(O_WDW, np.tile(wdw, (GROUPS, 1)))          # [128, 27]
    put(O_BQ, bq.reshape(CH, 1))
    put(O_BDW, np.tile(b_off_dw, GROUPS).reshape(CH, 1))
    put(O_LNW, np.tile(ln_w, GROUPS).reshape(CH, 1))
    put(O_LNWN, -np.tile(ln_w, GROUPS).reshape(CH, 1))
    put(O_LNB, np.tile(ln_b, GROUPS).reshape(CH, 1))
    put(O_YBO, (wo @ bv + bo).reshape(CH, 1))

    projw_neg = np.zeros((CH, 12), np.float32)
    for j in range(GROUPS):
        for ax in range(3):
            projw_neg[j * GC:(j + 1) * GC, ax * 4 + j] = -w_off_proj[ax]
    put(O_PROJ, projw_neg)

    blk = np.zeros((CH, GROUPS), np.float32)       # block membership
    for j in range(GROUPS):
        blk[j * GC:(j + 1) * GC, j] = 1.0
    put(O_MEAN, blk / GC)
    put(O_BCT, blk)                                # bcast lhsT, transposed
    bc4 = np.zeros((CH, GROUPS), np.float32)
    for j in range(GROUPS):
        bc4[32 * j + 1:32 * j + 17, j] = 1.0
    put(O_BC4, bc4)                                # denom bcast, transposed

    goff = np.zeros((CH, 1), np.float32)
    for j in range(GROUPS):
        goff[j, 0] = 1.0 + j * G_ROWS
    put(O_GOFF, goff)

    # sampling reference grid (z, y, x): rxyz[ax*4+j, s] identical over j
    r = (np.linspace(0.5, DK - 0.5, DK, dtype=np.float32) / DK) * 2 - 1
    zz, yy, xx = np.meshgrid(r, r, r, indexing="ij")
    axes = [zz.reshape(NS), yy.reshape(NS), xx.reshape(NS)]
    rxyz = np.zeros((12, NS), np.float32)
    for ax in range(3):
        for j in range(GROUPS):
            rxyz[ax * 4 + j] = (axes[ax] + 1.0) * 7.5 + 1.875
    rpk = np.zeros((CH, 48), np.float32)
    for rr in range(12):
        for q in range(4):
            rpk[:, rr * 4 + q] = rxyz[rr, q * 128:(q + 1) * 128]
    put(O_RXYZ, rpk)

    in_maps = []
    for c in range(NCORES):
        b, mq = c // 4, c % 4
        blob = np.zeros((CH, W_BLOB), np.float32)
        blob[:, 0:512] = _pack16(Qf[b][:, mq * QPC:(mq + 1) * QPC])
        blob[:, 512:1024] = _pack16(KVf[b][:, mq * QPC:(mq + 1) * QPC])
        blob[:, 1024:1024 + CHUNK] = com[:, mq * CHUNK:(mq + 1) * CHUNK]
        in_maps.append({"blob": blob})
    return in_maps


def host_post(results, bo=None):
    """results: list of 8 dicts with 'py16' [128, 1024] bf16."""
    y = np.zeros((B, CH, NQ), np.float32)
    for c in range(NCORES):
        b, mq = c // 4, c % 4
        y[b][:, mq * QPC:(mq + 1) * QPC] = np.asarray(
            results[c]["py16"]).astype(np.float32)
    return y.reshape(B, CH, SP, SP, SP)


# ============================================================ device build

def build_program(tc: tile.TileContext, ctx):
    nc = tc.nc

    blob = nc.dram_tensor("blob", [CH, W_BLOB], F32, kind="ExternalInput").ap()
    py16 = nc.dram_tensor("py16", [CH, QPC], BF16, kind="ExternalOutput").ap()

    kvt_dram = nc.dram_tensor("kvt_dram", [GROUPS * G_ROWS, 2 * GC], F32).ap()
    qkq_dram = nc.dram_tensor("qkq_dram", [CH, GCOL], F32).ap()
    qkall_dram = nc.dram_tensor("qkall_dram", [4 * CH, GCOL], F32).ap()
    idx_dram = nc.dram_tensor("idx_dram", [N_IDX], I16).ap()
    co_dram = nc.dram_tensor("co_dram", [12 * 3 * NS], F32).ap()
    w8_dram = nc.dram_tensor("w8_dram", [64 * 2 * CH], F32).ap()

    consts = ctx.enter_context(tc.tile_pool(name="consts", bufs=1))
    live = ctx.enter_context(tc.tile_pool(name="live", bufs=1))

    ident = consts.tile([CH, CH], F32, tag="ident", name="ident")
    make_identity(nc, ident[:])

    # reassembled shared weight block (filled from the AllGather result)
    com_sb = consts.tile([CH, COM_W], F32, tag="com_sb", name="com_sb")

    def bcol(off, n=1):
        return com_sb[:, off:off + n]

    C3 = 3 * CHUNK                    # first com col of replica-3's chunk

    def q3(off, n=1):                 # qkall view of a chunk-3 com column
        return qkall_dram[3 * CH:4 * CH, 1024 + off - C3:1024 + off - C3 + n]

    # [row-dim < 128] constant tiles; filled AFTER the AllGather below
    # (their com columns all live in replica 3's chunk of qkall)
    bcast_sb = consts.tile([GROUPS, CH], F32, tag="bcast_sb", name="bcast_sb")
    rxyz_sb = consts.tile([12, NS], F32, tag="rxyz_sb", name="rxyz_sb")
    goff_sb = consts.tile([GROUPS, 1], F32, tag="goff_sb", name="goff_sb")
    bq8 = consts.tile([HC, HEADS], F32, tag="bq8", name="bq8")
    bc4_sb = consts.tile([GROUPS, CH], F32, tag="bc4_sb", name="bc4_sb")

    # five bf16-packed [128,128] matrices -> f32 SBUF tiles
    w5 = consts.tile([CH, 5 * CH], F32, tag="w5", name="w5")
    wq_sb = w5[:, 0 * CH:1 * CH]
    wk_sb = w5[:, 1 * CH:2 * CH]
    wv_sb = w5[:, 2 * CH:3 * CH]
    woA_sb = w5[:, 3 * CH:4 * CH]
    woB_sb = w5[:, 4 * CH:5 * CH]

    convw_sb = consts.tile([CH, KS ** 3 * CH], F32, tag="convw_sb",
                           name="convw_sb")

    # tiles that outlive the scratch phases
    qsl = live.tile([CH, QPC], F32, tag="qsl", name="qsl")
    vt_sb = live.tile([CH, 4 * 2 * 4 * GC], F32, tag="vt_sb", name="vt_sb")
    xs_sb = live.tile([CH, GROUPS * CH], F32, tag="xs_sb", name="xs_sb")

    with tc.tile_pool(name="scratch", bufs=1) as scr, \
         tc.tile_pool(name="ps", bufs=2, space="PSUM") as ps_pool, \
         tc.tile_pool(name="pst", bufs=1, space="PSUM") as pst_pool:

        _cnt = [0]

        def st(tag, shape=(CH, NS), dt=F32):
            _cnt[0] += 1
            return scr.tile(list(shape), dt, tag=tag, name=f"{tag}_{_cnt[0]}")

        def ps512():
            return ps_pool.tile([CH, 512], F32, tag="ps512", name="ps512")

        # ---- phase 0: load blob, unpack, build conv diagonals --------
        blob_sb = st("blob", (CH, W_BLOB))
        nc.sync.dma_start(blob_sb[:], blob)

        # AllGather quarters of Q/KV + weight chunks within the batch group
        nc.sync.dma_start(qkq_dram, blob_sb[:, 0:GCOL])
        nc.gpsimd.collective_compute(
            "AllGather", ALU.bypass,
            replica_groups=[[0, 1, 2, 3], [4, 5, 6, 7]],
            ins=[qkq_dram], outs=[qkall_dram])
        qk_sb = st("qk", (CH, 4 * GCOL))
        nc.sync.dma_start(
            qk_sb[:].rearrange("p (r c) -> p r c", r=4),
            qkall_dram.rearrange("(r p) c -> p r c", r=4))
        qf_sb = st("qf", (CH, NQ))
        kv_sb = st("kv", (CH, NQ))
        for r in range(4):
            nc.scalar.activation(
                qf_sb[:, r * QPC:(r + 1) * QPC],
                qk_sb[:, r * GCOL:r * GCOL + 512].bitcast(BF16), AF.Identity)
            nc.scalar.activation(
                kv_sb[:, r * QPC:(r + 1) * QPC],
                qk_sb[:, r * GCOL + 512:r * GCOL + 1024].bitcast(BF16),
                AF.Identity)
            nc.vector.tensor_copy(
                com_sb[:, r * CHUNK:(r + 1) * CHUNK],
                qk_sb[:, r * GCOL + 1024:r * GCOL + GCOL])
        # this core's own 1024 queries (for attention Q)
        nc.scalar.activation(qsl[:], blob_sb[:, 0:512].bitcast(BF16),
                             AF.Identity)
        nc.sync.dma_start(bcast_sb[:], q3(O_BCT, 4).rearrange("p j -> j p"))
        nc.sync.dma_start(
            rxyz_sb[:].rearrange("r (q p) -> r q p", q=4),
            q3(O_RXYZ, 48).rearrange("p (r q) -> r q p", r=12))
        nc.sync.dma_start(goff_sb[:], q3(O_GOFF)[0:GROUPS, :])
        nc.sync.dma_start(
            bq8[:], q3(O_BQ).rearrange("(h c) j -> c (h j)", h=8))
        nc.sync.dma_start(bc4_sb[:], q3(O_BC4, 4).rearrange("p j -> j p"))
        nc.scalar.activation(
            w5[:], com_sb[:, O_W5:O_W5 + 5 * 64].bitcast(BF16), AF.Identity)
        for t in range(KS ** 3):
            nc.vector.tensor_scalar(convw_sb[:, t * CH:(t + 1) * CH],
                                    ident[:], bcol(O_WDW + t), None, ALU.mult)

        # transpose kv to spatial-major: tile k holds [sp 128k.., ch]
        kvT_sb = st("kvT", (CH, NQ))
        for k in range(NQ // CH):
            pt = ps512()
            nc.tensor.transpose(pt[:, 0:CH], kv_sb[:, k * CH:(k + 1) * CH],
                                ident[:])
            nc.scalar.activation(kvT_sb[:, k * CH:(k + 1) * CH], pt[:, 0:CH],
                                 AF.Identity)

        # scatter into the x-interleaved gather table
        # row (g, z, y, xs) = [KV[g,:,z,y,xs-1], KV[g,:,z,y,xs]]
        kvt_v = kvt_dram.rearrange("(g z y xs) w -> g z y xs w",
                                   g=GROUPS, z=SP, y=SP)
        for k in range(NQ // CH):
            z, yh = k // 2, k % 2
            for g in range(GROUPS):
                src = kvT_sb[:, k * CH + g * GC:k * CH + (g + 1) * GC]
                nc.sync.dma_start(
                    kvt_v[g, z, yh * 8:yh * 8 + 8, 1:SP + 1, 0:GC], src)
                nc.sync.dma_start(
                    kvt_v[g, z, yh * 8:yh * 8 + 8, 0:SP, GC:2 * GC], src)
        # finite-fill the two fetched-but-masked edge columns
        zfill = st("zfill", (CH, 256))
        nc.vector.memset(zfill[:], 0.0)
        nc.sync.dma_start(
            kvt_v[:, :, :, 0, 0:GC].rearrange("g z y c -> (g z y) c")
            .rearrange("(a b) c -> a b c", a=CH),
            zfill[:].rearrange("p (b c) -> p b c", c=GC))
        nc.sync.dma_start(
            kvt_v[:, :, :, SP, GC:2 * GC].rearrange("g z y c -> (g z y) c")
            .rearrange("(a b) c -> a b c", a=CH),
            zfill[:].rearrange("p (b c) -> p b c", c=GC))

        # ---- phase 1: Q projection -----------------------------------
        # full projection into a zero-padded 17^3 buffer (conv input)
        SPP = SP + 1
        q_pad = st("qpad", (CH, SPP ** 3))
        nc.gpsimd.memset(q_pad[:], 0.0)
        qp_zyx = q_pad[:].rearrange("p (z y x) -> p z y x", z=SPP, y=SPP)
        for i in range(NQ // 512):   # chunk = 2 z-slabs
            pq = ps512()
            nc.tensor.matmul(pq[:], wq_sb,
                             qf_sb[:, i * 512:(i + 1) * 512],
                             start=True, stop=True)
            nc.scalar.activation(
                qp_zyx[:, 1 + 2 * i:3 + 2 * i, 1:SP + 1, 1:SP + 1],
                pq[:].rearrange("p (a b c) -> p a b c", a=2, b=SP),
                AF.Identity, bias=bcol(O_BQ))

        # ---- phase 2: depthwise conv (stride 2) ----------------------
        pc = pst_pool.tile([CH, NS], F32, tag="psA", name="psA")
        first = True
        for dz in range(KS):
            for dy in range(KS):
                for dx in range(KS):
                    t = (dz * KS + dy) * KS + dx
                    rhs = qp_zyx[:, dz:dz + 2 * DK - 1:2,
                                 dy:dy + 2 * DK - 1:2,
                                 dx:dx + 2 * DK - 1:2]
                    nc.tensor.matmul(pc[:], convw_sb[:, t * CH:(t + 1) * CH],
                                     rhs, start=first,
                                     stop=(t == KS ** 3 - 1))
                    first = False
        c_sb = st("s0")
        nc.scalar.activation(c_sb[:], pc[:], AF.Identity, bias=bcol(O_BDW))

        # ---- phase 3: LayerNorm over 32-channel blocks ---------------
        csq = st("s1")
        nc.scalar.activation(csq[:], c_sb[:], AF.Square)
        pmu = pst_pool.tile([GROUPS, NS], F32, tag="psB", name="psB")
        nc.tensor.matmul(pmu[:], bcol(O_MEAN, 4), c_sb[:],
                         start=True, stop=True)
        pmsq = pst_pool.tile([GROUPS, NS], F32, tag="psC", name="psC")
        nc.tensor.matmul(pmsq[:], bcol(O_MEAN, 4), csq[:],
                         start=True, stop=True)
        mu2 = st("s2", (GROUPS, NS))
        nc.scalar.activation(mu2[:], pmu[:], AF.Square)
        var = st("s3", (GROUPS, NS))
        nc.vector.tensor_sub(var[:], pmsq[:], mu2[:])
        eps_sb = st("eps", (GROUPS, 1))
        nc.vector.memset(eps_sb[:], EPS)
        lnv = st("s2b", (GROUPS, NS))
        nc.scalar.activation(lnv[:], var[:], AF.Ln, bias=eps_sb[:])
        rstd = st("s1b", (GROUPS, NS))
        nc.scalar.activation(rstd[:], lnv[:], AF.Exp, scale=-0.5)
        murstd = st("s3b", (GROUPS, NS))
        nc.vector.tensor_mul(murstd[:], pmu[:], rstd[:])
        prb = pst_pool.tile([CH, NS], F32, tag="psA", name="psA2")
        nc.tensor.matmul(prb[:], bcast_sb[:], rstd[:], start=True, stop=True)
        pmb = pst_pool.tile([CH, NS], F32, tag="psA", name="psA3")
        nc.tensor.matmul(pmb[:], bcast_sb[:], murstd[:], start=True, stop=True)
        a_bc = st("s2")
        nc.vector.tensor_scalar(a_bc[:], prb[:], bcol(O_LNW), None, ALU.mult)
        b_bc = st("s3")
        nc.vector.tensor_scalar(b_bc[:], pmb[:], bcol(O_LNWN), bcol(O_LNB),
                                ALU.mult, ALU.add)
        u = st("s4")
        nc.vector.tensor_mul(u[:], c_sb[:], a_bc[:])
        nc.vector.tensor_add(u[:], u[:], b_bc[:])

        # ---- phase 4: gelu (tanh approx; tanh via exp) ---------------
        usq = st("s0")
        nc.scalar.activation(usq[:], u[:], AF.Square)
        ucb = st("s1")
        nc.vector.tensor_mul(ucb[:], usq[:], u[:])
        g2 = st("s2")
        nc.vector.scalar_tensor_tensor(g2[:], ucb[:], GELU_C, u[:],
                                       ALU.mult, ALU.add)
        ge = st("s3")
        nc.scalar.activation(ge[:], g2[:], AF.Exp, scale=2.0 * GELU_S)
        nc.vector.tensor_scalar(ge[:], ge[:], 1.0, None, ALU.add)
        gr = st("s0")
        nc.vector.reciprocal(gr[:], ge[:])
        gneg = st("s1")
        nc.vector.scalar_tensor_tensor(gneg[:], gr[:], 1.0, u[:],
                                       ALU.subtract, ALU.mult)  # -gelu

        # ---- phase 5: offset proj + coords ---------------------------
        poff = pst_pool.tile([12, NS], F32, tag="psB", name="psB2")
        nc.tensor.matmul(poff[:], bcol(O_PROJ, 12), gneg[:],
                         start=True, stop=True)
        ce = st("s2", (12, NS))
        nc.scalar.activation(ce[:], poff[:], AF.Exp, scale=2.0)
        nc.vector.tensor_scalar(ce[:], ce[:], 1.0, None, ALU.add)
        cr = st("s3", (12, NS))
        nc.vector.reciprocal(cr[:], ce[:])
        ixyz = st("s4", (12, NS))
        nc.vector.scalar_tensor_tensor(ixyz[:], cr[:], -3.75, rxyz_sb[:],
                                       ALU.mult, ALU.add)
        ci = st("s0", (12, NS), I32)
        nc.vector.tensor_copy(ci[:], ixyz[:])
        cf = st("s1", (12, NS))
        nc.vector.tensor_copy(cf[:], ci[:])
        fixm = st("s2", (12, NS))
        nc.vector.tensor_tensor(fixm[:], cf[:], ixyz[:], ALU.is_gt)
        f0 = st("s5", (12, NS))
        nc.vector.tensor_sub(f0[:], cf[:], fixm[:])
        tfrac = st("s3", (12, NS))
        nc.vector.tensor_sub(tfrac[:], ixyz[:], f0[:])
        m0 = st("s0", (12, NS))
        nc.vector.tensor_scalar(m0[:], f0[:], 0.0, None, ALU.is_ge)
        m1 = st("s1", (12, NS))
        nc.vector.tensor_scalar(m1[:], f0[:], 14.0, None, ALU.is_le)
        omt = st("s2", (12, NS))
        nc.vector.tensor_scalar(omt[:], tfrac[:], -1.0, 1.0, ALU.mult, ALU.add)

        big = st("big", (12, 3 * NS))
        nc.vector.tensor_copy(big[:, 0:NS], f0[:])
        nc.vector.tensor_mul(big[:, NS:2 * NS], omt[:], m0[:])
        nc.vector.tensor_mul(big[:, 2 * NS:3 * NS], tfrac[:], m1[:])
        nc.sync.dma_start(co_dram, big[:])
        co_g = st("co_g", (GROUPS, 9 * NS))
        nc.sync.dma_start(
            co_g[:].rearrange("g (ax k s) -> g ax k s", ax=3, k=3),
            co_dram.rearrange("(ax g k s) -> g ax k s", ax=3, g=4, k=3))

        def cgs(ax, kind):  # kind: 0 = floor, 1 = w0, 2 = w1
            o = (ax * 3 + kind) * NS
            return co_g[:, o:o + NS]

        zc0 = st("s0", (GROUPS, NS))
        zc1 = st("s1", (GROUPS, NS))
        yc0 = st("s2", (GROUPS, NS))
        yc1 = st("s3", (GROUPS, NS))
        nc.vector.tensor_scalar(zc0[:], cgs(0, 0), 0.0, 15.0, ALU.max, ALU.min)
        nc.vector.tensor_scalar(zc1[:], cgs(0, 0), 1.0, 0.0, ALU.add, ALU.max)
        nc.vector.tensor_scalar(zc1[:], zc1[:], 15.0, None, ALU.min)
        nc.vector.tensor_scalar(yc0[:], cgs(1, 0), 0.0, 15.0, ALU.max, ALU.min)
        nc.vector.tensor_scalar(yc1[:], cgs(1, 0), 1.0, 0.0, ALU.add, ALU.max)
        nc.vector.tensor_scalar(yc1[:], yc1[:], 15.0, None, ALU.min)
        xoff2 = st("s4", (GROUPS, NS))
        nc.vector.tensor_scalar(xoff2[:], cgs(2, 0), goff_sb[:], None, ALU.add)

        idxf = st("s5", (GROUPS, NS))
        idx16 = st("idx16", (GROUPS, 4 * NS), I16)
        wzy = st("wzy", (GROUPS, 4 * NS))
        zcs, ycs = [zc0, zc1], [yc0, yc1]
        for a in range(2):
            for bb in range(2):
                zy = a * 2 + bb
                nc.vector.scalar_tensor_tensor(
                    idxf[:], zcs[a][:], float(SP * XSLOTS), xoff2[:],
                    ALU.mult, ALU.add)
                nc.vector.scalar_tensor_tensor(
                    idxf[:], ycs[bb][:], float(XSLOTS), idxf[:],
                    ALU.mult, ALU.add)
                nc.vector.tensor_scalar(idxf[:], idxf[:], 0.0,
                                        float(GROUPS * G_ROWS - 1),
                                        ALU.max, ALU.min)
                nc.vector.tensor_copy(idx16[:, zy * NS:(zy + 1) * NS], idxf[:])
                nc.vector.tensor_mul(wzy[:, zy * NS:(zy + 1) * NS],
                                     cgs(0, 1 + a), cgs(1, 1 + bb))
        nc.sync.dma_start(idx_dram, idx16[:])
        # full trilinear corner weights w8[g, zy, x, s] = wzy * wx,
        # streamed piecewise to DRAM addressed (g zy si p x)
        w8d = w8_dram.rearrange("(g zy si p x) -> g zy si p x",
                                g=4, zy=4, si=4, x=2)
        for zy in range(4):
            for x in range(2):
                w8p = st("w8p", (GROUPS, NS))
                nc.vector.tensor_mul(w8p[:],
                                     wzy[:, zy * NS:(zy + 1) * NS],
                                     cgs(2, 1 + x))
                nc.sync.dma_start(
                    w8d[:, zy, :, :, x],
                    w8p[:].rearrange("g (si p) -> g si p", si=4))

        # wrapped idx [128, 512]: global idx i at (i%16, i//16), x8 blocks
        idxw = st("idxw", (CH, N_IDX // 16), I16)
        for rep in range(8):
            nc.gpsimd.dma_start(
                idxw[rep * 16:(rep + 1) * 16, :],
                idx_dram.rearrange("(col r) -> r col", r=16))

        # ---- phase 6: gather + trilinear combine ---------------------
        gth = scr.tile([CH, N_IDX // CH, 2 * GC], F32, tag="gth", name="gth")
        NCHK = 32
        CH_I = N_IDX // NCHK          # 256 idx per gather call
        for k in range(NCHK):
            nc.gpsimd.dma_gather(
                out_ap=gth[:, k * (CH_I // 128):(k + 1) * (CH_I // 128), :],
                in_ap=kvt_dram,
                idxs_ap=idxw[:, k * (CH_I // 16):(k + 1) * (CH_I // 16)],
                num_idxs=CH_I, num_idxs_reg=CH_I, elem_size=2 * GC)

        # stream order: i = ((g*4 + zy)*4 + si)*128 + p, sample s = si*128+p
        w8b = scr.tile([CH, 64, 2], F32, tag="tB", name="w8b")
        nc.sync.dma_start(
            w8b[:],
            w8_dram.rearrange("(j p x) -> p j x", j=64, x=2))
        nc.vector.tensor_tensor(
            gth[:].rearrange("p j (x c) -> p j x c", x=2),
            gth[:].rearrange("p j (x c) -> p j x c", x=2),
            w8b[:].unsqueeze(3).broadcast_to([CH, 64, 2, GC]), ALU.mult)
        t2v = gth[:].rearrange("p (g zy si) e -> p g zy (si e)", g=4, zy=4)
        sa = st("sa", (CH, GROUPS, 4 * 2 * GC))
        sb = st("sb", (CH, GROUPS, 4 * 2 * GC))
        nc.vector.tensor_tensor(sa[:], t2v[:, :, 0], t2v[:, :, 1], ALU.add)
        nc.vector.tensor_tensor(sb[:], t2v[:, :, 2], t2v[:, :, 3], ALU.add)
        nc.vector.tensor_tensor(sa[:], sa[:], sb[:], ALU.add)
        sav = sa[:].rearrange("p g (si x c) -> p g si x c", si=4, x=2)
        xs_t = st("s0", (CH, 4, GROUPS, GC))   # [p, si, g, c]
        nc.vector.tensor_tensor(xs_t[:].rearrange("p si g c -> p g si c"),
                                sav[:, :, :, 0, :],
                                sav[:, :, :, 1, :], ALU.add)

        # ---- phase 7: transpose to xs [128 (g,c), 512 n] -------------
        for si in range(4):
            pt = ps512()
            nc.tensor.transpose(
                pt[:, 0:CH], xs_t[:, si].rearrange("p g c -> p (g c)"),
                ident[:])
            nc.scalar.activation(xs_sb[:, si * CH:(si + 1) * CH], pt[:, 0:CH],
                                 AF.Identity)

        # ---- phase 8: V-hat ------------------------------------------
        nc.vector.memset(vt_sb[:], 0.0)
        nc.vector.memset(
            vt_sb[:].rearrange("p (n h s) -> p n h s", n=4, h=HEADS)
            [:, :, :, 0:1], 1.0)
        for nch in range(4):
            pv = ps512()
            nc.tensor.matmul(pv[:, 0:CH],
                             xs_sb[:, nch * CH:(nch + 1) * CH],
                             wv_sb, start=True, stop=True)
            nc.vector.tensor_copy(
                vt_sb[:].rearrange("p (n h s) -> p n h s", n=4, h=HEADS)
                [:, nch, :, 1:HC + 1],
                pv[:, 0:CH].rearrange("p (h c) -> p h c", h=HEADS))
        # (vt layout per n-chunk: 8 x [1 | V(16) | 0*15], 256 wide)

    # ---- phase 9: attention loop -------------------------------------
    # per query-half: Q/K per head on the fly, logits -> exp -> V-hat
    # accumulation (denominator in row 0 of each 32-block), normalize,
    # project through wo and emit bf16.
    with tc.tile_pool(name="pA", bufs=2, space="PSUM") as pA, \
         tc.tile_pool(name="pO", bufs=2, space="PSUM") as pO, \
         tc.tile_pool(name="pR", bufs=1, space="PSUM") as pR, \
         tc.tile_pool(name="pY", bufs=1, space="PSUM") as pY, \
         tc.tile_pool(name="att_pool", bufs=3) as att_pool, \
         tc.tile_pool(name="opool", bufs=2) as opool:
        for qh in range(2):
            on_tiles = {}
            for AB in range(2):
                po = pO.tile([CH, 512], F32, tag="po", name="po")
                for h4 in range(4):
                    h = AB * 4 + h4
                    pq2 = pA.tile([HC, 512], F32, tag="p16", name="p16q")
                    nc.tensor.matmul(pq2[:], wq_sb[:, HC * h:HC * (h + 1)],
                                     qsl[:, qh * 512:(qh + 1) * 512],
                                     start=True, stop=True)
                    q2h = att_pool.tile([HC, 512], F32, tag="q2h", name="q2h")
                    nc.vector.tensor_scalar(q2h[:], pq2[:], bq8[:, h:h + 1],
                                            None, ALU.add)
                    pk = pA.tile([HC, 512], F32, tag="p16", name="p16k")
                    nc.tensor.matmul(pk[:], wk_sb[:, HC * h:HC * (h + 1)],
                                     xs_sb[:], start=True, stop=True)
                    k2h = att_pool.tile([HC, 512], F32, tag="k2h", name="k2h")
                    nc.scalar.activation(k2h[:], pk[:], AF.Identity)
                    for nch in range(4):
                        pa = pA.tile([CH, 512], F32, tag="pa", name="pa")
                        nc.tensor.matmul(
                            pa[:], k2h[:, nch * CH:(nch + 1) * CH], q2h[:],
                            start=True, stop=True)
                        att = att_pool.tile([CH, 512], F32, tag="att",
                                            name="att")
                        nc.scalar.activation(att[:], pa[:], AF.Exp)
                        nc.tensor.matmul(
                            po[32 * h4:32 * h4 + 32, :],
                            vt_sb[:, nch * 256 + h * 32:
                                  nch * 256 + h * 32 + 32],
                            att[:], start=(nch == 0), stop=(nch == 3),
                            skip_group_check=True,
                            tile_position=(0, 32 * h4))
                o_sb = opool.tile([CH, 512], F32, tag="o_sb", name="o_sb")
                nc.scalar.activation(o_sb[:], po[:], AF.Identity)
                den4 = opool.tile([GROUPS, 512], F32, tag="den4", name="den4")
                for j in range(4):
                    nc.sync.dma_start(den4[j:j + 1, :],
                                      o_sb[32 * j:32 * j + 1, :])
                rd4 = opool.tile([GROUPS, 512], F32, tag="rd4", name="rd4")
                nc.vector.reciprocal(rd4[:], den4[:])
                prd = pR.tile([CH, 512], F32, tag="prd", name="prd")
                nc.tensor.matmul(prd[:], bc4_sb[:], rd4[:],
                                 start=True, stop=True)
                on_sb = opool.tile([CH, 512], F32, tag=f"on{AB}",
                                   name=f"on{AB}")
                nc.vector.tensor_mul(on_sb[:], o_sb[:], prd[:])
                on_tiles[AB] = on_sb

            pyp = pY.tile([CH, 512], F32, tag="pyp", name="pyp")
            for AB in range(2):
                nc.tensor.matmul(pyp[:],
                                 (woA_sb if AB == 0 else woB_sb),
                                 on_tiles[AB][:],
                                 start=(AB == 0), stop=(AB == 1))
            y16 = opool.tile([CH, 512], BF16, tag="y16", name="y16")
            nc.scalar.activation(y16[:], pyp[:], AF.Identity,
                                 bias=bcol(O_YBO))
            nc.sync.dma_start(py16[:, qh * 512:(qh + 1) * 512], y16[:])


# ============================================================ entry points

_CACHE = {}


def _get_compiled():
    if "nc" in _CACHE:
        return _CACHE["nc"]
    from contextlib import ExitStack
    nc = bacc.Bacc("TRN2", target_bir_lowering=False, debug=False,
                   num_devices=NCORES)
    with tile.TileContext(nc) as tc:
        with ExitStack() as ctx:
            build_program(tc, ctx)
    nc.compile()
    _CACHE["nc"] = nc
    return nc


def _get_dispatch():
    """A cached PJRT dispatcher: same semantics as
    bass2jax.run_bass_via_pjrt (fresh host inputs in, numpy outputs back),
    but the jitted shard_map callable is built ONCE and the donated output
    buffers are created on-device instead of being shipped through the
    axon tunnel every call."""
    if "dispatch" in _CACHE:
        return _CACHE["dispatch"]
    import jax
    import jax.numpy as jnp
    from jax.sharding import Mesh, PartitionSpec, NamedSharding
    from jax.experimental.shard_map import shard_map
    from concourse.bass2jax import (_bass_exec_p, install_neuronx_cc_hook,
                                    partition_id_tensor)

    nc = _get_compiled()
    install_neuronx_cc_hook()
    n_cores = NCORES
    partition_name = (nc.partition_id_tensor.name
                      if nc.partition_id_tensor else None)
    in_names, out_names, out_avals = [], [], []
    for alloc in nc.m.functions[0].allocations:
        if not isinstance(alloc, mybir.MemoryLocationSet):
            continue
        name = alloc.memorylocations[0].name
        if alloc.kind == "ExternalInput":
            if name != partition_name:
                in_names.append(name)
        elif alloc.kind == "ExternalOutput":
            out_names.append(name)
            out_avals.append(jax.core.ShapedArray(
                tuple(alloc.tensor_shape), mybir.dt.np(alloc.dtype)))
    n_params = len(in_names)
    n_outs = len(out_avals)
    all_in_names = in_names + out_names
    if partition_name is not None:
        all_in_names.append(partition_name)

    def _body(*args):
        operands = list(args)
        if partition_name is not None:
            operands.append(partition_id_tensor())
        outs = _bass_exec_p.bind(
            *operands, out_avals=tuple(out_avals),
            in_names=tuple(all_in_names), out_names=tuple(out_names),
            lowering_input_output_aliases=(),
            sim_require_finite=True, sim_require_nnan=True, nc=nc)
        return tuple(outs)

    devices = jax.devices()[:n_cores]
    mesh = Mesh(np.asarray(devices), ("core",))
    in_specs = (PartitionSpec("core"),) * (n_params + n_outs)
    out_specs = (PartitionSpec("core"),) * n_outs
    donate = tuple(range(n_params, n_params + n_outs))
    sharded = jax.jit(
        shard_map(_body, mesh=mesh, in_specs=in_specs,
                  out_specs=out_specs, check_rep=False),
        donate_argnums=donate, keep_unused=True)

    shard = NamedSharding(mesh, PartitionSpec("core"))
    zmaker = jax.jit(
        lambda: tuple(
            jnp.zeros((n_cores * a.shape[0], *a.shape[1:]), a.dtype)
            for a in out_avals),
        out_shardings=(shard,) * n_outs)

    def dispatch(in_maps):
        concat_in = [
            np.concatenate([np.asarray(m[nm]) for m in in_maps], axis=0)
            for nm in in_names]
        out_arrs = sharded(*concat_in, *zmaker())
        return [
            {nm: np.asarray(out_arrs[i]).reshape(
                n_cores, *out_avals[i].shape)[c]
             for i, nm in enumerate(out_names)}
            for c in range(n_cores)]

    _CACHE["dispatch"] = dispatch
    return dispatch


def kernel(**inputs):
    dispatch = _get_dispatch()
    in_maps = host_prep(inputs)
    res = dispatch(in_maps)
    return host_post(res, inputs.get("bo"))


if __name__ == "__main__":
    _get_compiled()
    print("build + compile OK")
